# revision 5
# baseline (speedup 1.0000x reference)
"""MLA (multi-head latent attention) prefill kernel for 8 TRN2 NeuronCores.

Sharding: tensor-parallel over heads (16 heads -> 2 per core). wq / wkv_b /
wo are sliced per head on the host; the kv_a latent projection is replicated.
Each core computes a partial output projection (its heads' contribution
through wo); the host sums the 8 partials.

Device layout notes (all matmuls bf16, fp32 PSUM accumulation):
 - x is transposed on the host to xT [2048, 4096] so every projection runs
   channel-major: out[c, s] with weights as the stationary operand.
 - Per-head qk channel order is [rope_lo(32); rope_hi(32); nope(64)] with the
   rope pairs deinterleaved on the host (wq / wkv_a rows permuted). RoPE then
   only combines partition ranges [0:32] x [32:64] straight out of PSUM,
   which the hardware allows (PSUM operands are exempt from the equal-base-
   partition rule).
 - Scores are computed k-major: st[k, q] = (k_tile)^T q. Softmax sums over k
   (partitions) via a ones-column matmul; exp runs on the ACT engine reading
   PSUM directly, with the 1/sqrt(d) scale folded in, writing bf16 probs.
 - The additive mask is folded in with an identity-weight matmul accumulated
   into the score PSUM, only for mask blocks that are not all-zero. Blocks
   whose mask is <= -1e8 everywhere (causal upper triangle) are skipped
   entirely (exp underflows to exactly 0 in the reference as well).
 - attention output is produced transposed [dv, q]; wo consumes it directly
   and the partial output is written [m, s]; host transposes once.
"""

import os
import sys

sys.path.insert(0, "/opt/trn_rl_repo")

import numpy as np
import ml_dtypes

import concourse.bass as bass
import concourse.tile as tile
import concourse.mybir as mybir
from concourse import bacc
from concourse.bass_utils import run_bass_kernel_spmd
from concourse.masks import make_identity

BF16 = mybir.dt.bfloat16
F32 = mybir.dt.float32
NPBF16 = ml_dtypes.bfloat16

S = 4096          # sequence length
D = 2048          # model dim
H = 16            # total heads
HPC = 2           # heads per core
NCORES = 8
L = 1024          # kv lora rank
ROPE = 64
NOPE = 64
VH = 128          # v head dim
SCALE = 128.0 ** -0.5
EPS = 1e-6

SB = 512          # free-dim block size
NSB = S // SB     # 8
NE = D // 128     # 16 e-chunks
NL = L // 128     # 8 latent chunks
NKT = S // 128    # 32 k tiles

last_results = None   # BassKernelResults of the most recent run (for test.py)

_BUILD_CACHE: dict = {}


def _bcast128(ap, n):
    """[1, n] DRAM AP -> [128, n] stride-0 partition broadcast AP."""
    return bass.AP(tensor=ap.tensor, offset=ap.offset, ap=[[0, 128], [1, n]])


def _build(skip, add):
    """Build + schedule the per-core Bass program.

    skip/add: [NKT][NSB] bool grids over (k-tile, q-block) mask blocks.
    """
    nc = bacc.Bacc("TRN2", target_bir_lowering=False, debug=False,
                   num_devices=NCORES)

    need_mask = bool(np.asarray(add).any())

    xT_d = nc.dram_tensor("xT", [D, S], BF16, kind="ExternalInput")
    cosT_d = nc.dram_tensor("cosT", [32, S], F32, kind="ExternalInput")
    sinT_d = nc.dram_tensor("sinT", [32, S], F32, kind="ExternalInput")
    wqT_d = nc.dram_tensor("wqT", [D, 128 * HPC], BF16, kind="ExternalInput")
    wkvaT_d = nc.dram_tensor("wkvaT", [D, L + ROPE], BF16, kind="ExternalInput")
    wkvbk_d = nc.dram_tensor("wkvbTk", [L, NOPE * HPC], BF16, kind="ExternalInput")
    wkvbv_d = nc.dram_tensor("wkvbTv", [L, VH * HPC], BF16, kind="ExternalInput")
    woT_d = nc.dram_tensor("woT", [VH * HPC, D], BF16, kind="ExternalInput")
    if need_mask:
        maskT_d = nc.dram_tensor("maskT", [S, S], BF16, kind="ExternalInput")
    out_d = nc.dram_tensor("out", [D, S], F32, kind="ExternalOutput")

    xT_r = xT_d[:].rearrange("(eo p) s -> p eo s", p=128)

    with tile.TileContext(nc) as tc:
        with (
            tc.tile_pool(name="singles", bufs=1) as singles,
            tc.tile_pool(name="persist", bufs=1) as persist,
        ):
            wq_s = singles.tile([128, NE, 128 * HPC], BF16)
            nc.sync.dma_start(wq_s[:], wqT_d[:].rearrange("(eo p) c -> p eo c", p=128))
            wkva_s = singles.tile([128, NE, L + ROPE], BF16)
            nc.sync.dma_start(wkva_s[:], wkvaT_d[:].rearrange("(eo p) c -> p eo c", p=128))
            wkvbk_s = singles.tile([128, NL, NOPE * HPC], BF16)
            nc.sync.dma_start(wkvbk_s[:], wkvbk_d[:].rearrange("(lo p) c -> p lo c", p=128))
            wkvbv_s = singles.tile([128, NL, VH * HPC], BF16)
            nc.sync.dma_start(wkvbv_s[:], wkvbv_d[:].rearrange("(lo p) c -> p lo c", p=128))
            wo_s = singles.tile([128, HPC, D], BF16)
            nc.sync.dma_start(wo_s[:], woT_d[:].rearrange("(co p) m -> p co m", p=128))
            ident = singles.tile([128, 128], BF16)
            make_identity(nc, ident[:])
            ones_c = singles.tile([128, 1], BF16)
            nc.vector.memset(ones_c[:], 1.0)
            eps_t = singles.tile([1, 1], F32)
            nc.vector.memset(eps_t[:], EPS)

            q_all = persist.tile([128, HPC, S], BF16)   # per head: [pe_lo;pe_hi;nope]
            k0 = persist.tile([128, S], BF16)
            k1 = persist.tile([128, S], BF16)
            v_sb = persist.tile([128, NKT, VH * HPC], BF16)  # s-major v
            attn_T = persist.tile([128, HPC, S], BF16)  # [dv, s] per head

            # ---------------- phase A+B: projections -----------------
            with (
                tc.tile_pool(name="abx", bufs=2) as abx,
                tc.tile_pool(name="abw", bufs=2) as abw,
                tc.tile_pool(name="ab1", bufs=1) as ab1,
                tc.tile_pool(name="rp", bufs=2) as rp,
                tc.tile_pool(name="psA", bufs=4, space="PSUM") as psA,
                tc.tile_pool(name="drA", bufs=2, space="DRAM") as drA,
            ):
                def rope(ps_pe, cos_t, sin_t, out_lo, out_hi):
                    m1 = rp.tile([32, SB], F32, tag="m1")
                    m2 = rp.tile([32, SB], F32, tag="m2")
                    m3 = rp.tile([32, SB], F32, tag="m3")
                    m4 = rp.tile([32, SB], F32, tag="m4")
                    nc.vector.tensor_mul(m1[:], ps_pe[0:32, :], cos_t[:])
                    nc.vector.tensor_mul(m2[:], ps_pe[32:64, :], sin_t[:])
                    nc.vector.tensor_mul(m3[:], ps_pe[0:32, :], sin_t[:])
                    nc.vector.tensor_mul(m4[:], ps_pe[32:64, :], cos_t[:])
                    nc.vector.tensor_sub(out_lo, m1[:], m2[:])
                    nc.vector.tensor_add(out_hi, m3[:], m4[:])

                for sb in range(NSB):
                    ssl = slice(sb * SB, (sb + 1) * SB)
                    x_t = abx.tile([128, NE, SB], BF16, tag="x")
                    nc.sync.dma_start(x_t[:], xT_r[:, :, ssl])
                    cos_t = abw.tile([32, SB], F32, tag="cos")
                    nc.sync.dma_start(cos_t[:], cosT_d[:, ssl])
                    sin_t = abw.tile([32, SB], F32, tag="sin")
                    nc.sync.dma_start(sin_t[:], sinT_d[:, ssl])

                    # q projection (2 head-tiles), rope applied from PSUM
                    for ct in range(HPC):
                        qp = psA.tile([128, SB], F32, tag="ps")
                        for e in range(NE):
                            nc.tensor.matmul(qp[:], wq_s[:, e, ct * 128:(ct + 1) * 128],
                                             x_t[:, e, :], start=(e == 0), stop=(e == NE - 1))
                        nc.scalar.copy(q_all[64:128, ct, ssl], qp[64:128, :])
                        rope(qp, cos_t, sin_t,
                             q_all[0:32, ct, ssl], q_all[32:64, ct, ssl])

                    # kv_a latent projection + squares
                    sq_t = ab1.tile([128, NL, SB], BF16, tag="sq")
                    lat_t = ab1.tile([128, NL, SB], BF16, tag="lat")
                    for lt in range(NL):
                        lp = psA.tile([128, SB], F32, tag="ps")
                        for e in range(NE):
                            nc.tensor.matmul(lp[:], wkva_s[:, e, lt * 128:(lt + 1) * 128],
                                             x_t[:, e, :], start=(e == 0), stop=(e == NE - 1))
                        nc.scalar.activation(sq_t[:, lt, :], lp[:],
                                             mybir.ActivationFunctionType.Square)
                        nc.vector.tensor_copy(lat_t[:, lt, :], lp[:])

                    # k_pe projection + rope -> k0 rows 0:64 (shared by k1)
                    kp = psA.tile([64, SB], F32, tag="ps")
                    for e in range(NE):
                        nc.tensor.matmul(kp[:], wkva_s[:, e, L:L + ROPE],
                                         x_t[:, e, :], start=(e == 0), stop=(e == NE - 1))
                    rope(kp, cos_t, sin_t, k0[0:32, ssl], k0[32:64, ssl])
                    nc.vector.tensor_copy(k1[0:64, ssl], k0[0:64, ssl])

                    # rmsnorm scale g = rsqrt(mean(latent^2) + eps)
                    sp = psA.tile([1, SB], F32, tag="ps")
                    for lt in range(NL):
                        nc.tensor.matmul(sp[:], ones_c[:], sq_t[:, lt, :],
                                         start=(lt == 0), stop=(lt == NL - 1))
                    rs_t = abw.tile([1, SB], F32, tag="rs")
                    nc.scalar.activation(rs_t[:], sp[:], mybir.ActivationFunctionType.Sqrt,
                                         bias=eps_t[:], scale=1.0 / L)
                    g_t = abw.tile([1, SB], F32, tag="g")
                    nc.vector.reciprocal(g_t[:], rs_t[:])
                    g_dram = drA.tile([1, SB], F32, tag="gd")
                    nc.sync.dma_start(g_dram[:], g_t[:])
                    gb_t = abw.tile([128, SB], F32, tag="gb")
                    nc.sync.dma_start(gb_t[:], _bcast128(g_dram[:], SB))
                    for lt in range(NL):
                        nc.vector.tensor_mul(lat_t[:, lt, :], lat_t[:, lt, :], gb_t[:])

                    # kv_b k_nope: psum rows 0:64 -> k0[64:128], 64:128 -> k1[64:128]
                    kbp = psA.tile([128, SB], F32, tag="ps")
                    for lt in range(NL):
                        nc.tensor.matmul(kbp[:], wkvbk_s[:, lt, :], lat_t[:, lt, :],
                                         start=(lt == 0), stop=(lt == NL - 1))
                    nc.vector.tensor_copy(k0[64:128, ssl], kbp[0:64, :])
                    nc.vector.tensor_copy(k1[64:128, ssl], kbp[64:128, :])

                    # kv_b v (s-major): lhsT = latent tile, rhs = wkvb_v
                    for st in range(SB // 128):
                        vp = psA.tile([128, VH * HPC], F32, tag="ps")
                        for lt in range(NL):
                            nc.tensor.matmul(vp[:], lat_t[:, lt, st * 128:(st + 1) * 128],
                                             wkvbv_s[:, lt, :],
                                             start=(lt == 0), stop=(lt == NL - 1))
                        nc.scalar.copy(v_sb[:, sb * 4 + st, :], vp[:])

            # ---------------- phase C: attention -----------------
            with (
                tc.tile_pool(name="mp", bufs=3) as mp,
                tc.tile_pool(name="pb", bufs=1) as pb,
                tc.tile_pool(name="dvp", bufs=2) as dvp,
                tc.tile_pool(name="ost", bufs=3) as ost,
                tc.tile_pool(name="psatt", bufs=4, space="PSUM") as psatt,
                tc.tile_pool(name="psden", bufs=2, space="PSUM") as psden,
                tc.tile_pool(name="pso", bufs=2, space="PSUM") as pso,
                tc.tile_pool(name="drC", bufs=2, space="DRAM") as drC,
            ):
                for h in range(HPC):
                    kh = k0 if h == 0 else k1
                    for qb in range(NSB):
                        qsl = slice(qb * SB, (qb + 1) * SB)
                        active = [ki for ki in range(NKT) if not skip[ki][qb]]
                        probsT = pb.tile([128, NKT, SB], BF16, tag="probsT")
                        for ki in active:
                            ap_ = psatt.tile([128, SB], F32, tag="att")
                            has_mask = add[ki][qb]
                            nc.tensor.matmul(ap_[:], kh[:, ki * 128:(ki + 1) * 128],
                                             q_all[:, h, qsl],
                                             start=True, stop=not has_mask)
                            if has_mask:
                                m_t = mp.tile([128, SB], BF16, tag="mask")
                                nc.sync.dma_start(
                                    m_t[:], maskT_d[ki * 128:(ki + 1) * 128, qsl])
                                nc.tensor.matmul(ap_[:], ident[:], m_t[:],
                                                 start=False, stop=True)
                            nc.scalar.activation(probsT[:, ki, :], ap_[:],
                                                 mybir.ActivationFunctionType.Exp,
                                                 scale=SCALE)
                        dp = psden.tile([1, SB], F32, tag="den")
                        for i, ki in enumerate(active):
                            nc.tensor.matmul(dp[:], ones_c[:], probsT[:, ki, :],
                                             start=(i == 0), stop=(i == len(active) - 1))
                        dinv = dvp.tile([1, SB], F32, tag="dinv")
                        nc.vector.reciprocal(dinv[:], dp[:])
                        dv_dram = drC.tile([1, SB], F32, tag="dvd")
                        nc.sync.dma_start(dv_dram[:], dinv[:])
                        dinvb = dvp.tile([128, SB], F32, tag="dinvb")
                        nc.sync.dma_start(dinvb[:], _bcast128(dv_dram[:], SB))
                        op_ = pso.tile([128, SB], F32, tag="o")
                        for i, ki in enumerate(active):
                            nc.tensor.matmul(op_[:], v_sb[:, ki, h * VH:(h + 1) * VH],
                                             probsT[:, ki, :],
                                             start=(i == 0), stop=(i == len(active) - 1))
                        nc.vector.tensor_mul(attn_T[:, h, qsl], op_[:], dinvb[:])

                # ---------------- phase D: output projection -----------------
                for mt in range(D // 128):
                    for s2 in range(NSB):
                        wp = psatt.tile([128, SB], F32, tag="att")
                        for cc in range(HPC):
                            nc.tensor.matmul(wp[:], wo_s[:, cc, mt * 128:(mt + 1) * 128],
                                             attn_T[:, cc, s2 * SB:(s2 + 1) * SB],
                                             start=(cc == 0), stop=(cc == HPC - 1))
                        o_t = ost.tile([128, SB], F32, tag="ostage")
                        nc.any.tensor_copy(out=o_t[:], in_=wp[:])
                        nc.sync.dma_start(
                            out_d[mt * 128:(mt + 1) * 128, s2 * SB:(s2 + 1) * SB],
                            o_t[:])

    nc.compile()
    return nc, need_mask


def kernel(x, cos, sin, mask, wq, wkv_a, kv_norm_w, wkv_b, wo, start_pos=0):
    x = np.asarray(x, np.float32)
    cos = np.asarray(cos, np.float32)
    sin = np.asarray(sin, np.float32)
    mask = np.asarray(mask, np.float32)
    wq = np.asarray(wq, np.float32)
    wkv_a = np.asarray(wkv_a, np.float32)
    kv_norm_w = np.asarray(kv_norm_w, np.float32)
    wkv_b = np.asarray(wkv_b, np.float32)
    wo = np.asarray(wo, np.float32)

    # mask block metadata: [qb, qi, kt, kj]
    mr = mask.reshape(NSB, SB, NKT, 128)
    skip_qk = (mr <= -1e8).all(axis=(1, 3))          # [qb, kt]
    nonzero_qk = (mr != 0).any(axis=(1, 3))          # [qb, kt]
    skip = skip_qk.T.copy()                          # [kt, qb]
    add = (nonzero_qk & ~skip_qk).T.copy()
    key = (skip.tobytes(), add.tobytes())
    if key not in _BUILD_CACHE:
        _BUILD_CACHE[key] = _build(skip, add)
    nc, need_mask = _BUILD_CACHE[key]

    # ---- host-side shard prep ----
    deint = np.concatenate([np.arange(0, ROPE, 2), np.arange(1, ROPE, 2)])
    wq_h = wq.reshape(H, 128, D)
    # per-head row order [rope deinterleaved; nope]
    qrows = np.concatenate([wq_h[:, NOPE + deint, :], wq_h[:, 0:NOPE, :]], axis=1)
    wkva_perm = np.concatenate([wkv_a[0:L], wkv_a[L + deint]], axis=0)
    wkvb_h = wkv_b.reshape(H, NOPE + VH, L)

    xT = np.ascontiguousarray(x[0].T).astype(NPBF16)
    cosT = np.ascontiguousarray(cos.T)
    sinT = np.ascontiguousarray(sin.T)
    wkvaT = np.ascontiguousarray(wkva_perm.T).astype(NPBF16)
    shared = {"xT": xT, "cosT": cosT, "sinT": sinT, "wkvaT": wkvaT}
    if need_mask:
        shared["maskT"] = np.ascontiguousarray(mask.T * (1.0 / SCALE)).astype(NPBF16)

    in_maps = []
    for c in range(NCORES):
        hs = [HPC * c + i for i in range(HPC)]
        wqT_c = np.ascontiguousarray(
            qrows[hs].reshape(128 * HPC, D).T).astype(NPBF16)
        k_rows = (wkvb_h[hs, 0:NOPE, :] * kv_norm_w[None, None, :]).reshape(
            NOPE * HPC, L)
        wkvbTk_c = np.ascontiguousarray(k_rows.T).astype(NPBF16)
        v_rows = wkvb_h[hs, NOPE:, :].reshape(VH * HPC, L)
        wkvbTv_c = np.ascontiguousarray(v_rows.T).astype(NPBF16)
        woT_c = np.ascontiguousarray(
            wo[:, hs[0] * VH:(hs[-1] + 1) * VH].T).astype(NPBF16)
        m = dict(shared)
        m.update({"wqT": wqT_c, "wkvbTk": wkvbTk_c, "wkvbTv": wkvbTv_c,
                  "woT": woT_c})
        in_maps.append(m)

    trace = os.environ.get("KERNEL_TRACE", "0") == "1"
    if trace:
        _install_ntff_hook()
    global last_results
    last_results = run_bass_kernel_spmd(nc, in_maps, core_ids=list(range(NCORES)),
                                        trace=trace)
    total = np.zeros((D, S), np.float32)
    for r in last_results.results:
        total += r["out"]
    return np.ascontiguousarray(total.T)[None]


def _install_ntff_hook():
    """Register the axon NTFF profiling hook (used when KERNEL_TRACE=1)."""
    import types
    import ctypes
    import contextlib

    if "antenv.axon_hooks" in sys.modules:
        return
    try:
        so = ctypes.CDLL("/opt/axon/libaxon_pjrt.so")
        so.axon_start_nrt_profile
    except (OSError, AttributeError):
        return
    so.axon_start_nrt_profile.argtypes = [ctypes.POINTER(ctypes.c_int64),
                                          ctypes.c_size_t]
    so.axon_start_nrt_profile.restype = ctypes.c_int64
    so.axon_stop_nrt_profile.argtypes = [ctypes.c_char_p]
    so.axon_stop_nrt_profile.restype = ctypes.c_int64

    @contextlib.contextmanager
    def _hook(output_dir, device_ids):
        import jax
        jax.devices()
        if device_ids:
            ids = (ctypes.c_int64 * len(device_ids))(*device_ids)
            rc = so.axon_start_nrt_profile(ids, len(device_ids))
        else:
            rc = so.axon_start_nrt_profile(None, 0)
        if rc != 0:
            raise RuntimeError(f"axon_start_nrt_profile rc={rc}")
        try:
            yield
        finally:
            n = so.axon_stop_nrt_profile(str(output_dir).encode())
            if n < 0:
                raise RuntimeError(f"axon_stop_nrt_profile rc={n}")

    mod = types.ModuleType("antenv.axon_hooks")
    mod.get_axon_ntff_profile_hook = lambda: _hook
    mod.set_axon_ntff_profile_hook = lambda h: None
    sys.modules["antenv.axon_hooks"] = mod


# revision 13
# speedup vs baseline: 1.1830x; 1.1830x over previous
"""MLA (multi-head latent attention) prefill kernel for 8 TRN2 NeuronCores.

Sharding: tensor-parallel over heads (16 heads -> 2 per core). wq / wkv_b /
wo are sliced per head on the host. The kv_a latent projection is sharded
over the sequence (each core computes 512 positions), rms-normalized and
rope'd locally, then AllGathered on-device so every core holds the full
normalized latent + k_pe. Each core computes a partial output projection
(its heads' contribution through wo); the host sums the 8 partials.

Device layout notes (all matmuls bf16, fp32 PSUM accumulation):
 - x is transposed on the host to xT [2048, 4096] so every projection runs
   channel-major: out[c, s] with weights as the stationary operand.
 - Per-head qk channel order is [rope_lo(32); rope_hi(32); nope(64)] with the
   rope pairs deinterleaved on the host (wq / wkv_a rows permuted). RoPE then
   only combines partition ranges [0:32] x [32:64] straight out of PSUM,
   which the hardware allows (PSUM operands are exempt from the equal-base-
   partition rule).
 - Scores are computed k-major: st[k, q] = (k_tile)^T q. Softmax sums over k
   (partitions) via a ones-column matmul; exp runs on the ACT engine reading
   PSUM directly, with the 1/sqrt(d) scale folded in, writing bf16 probs.
 - The additive mask is folded in with an identity-weight matmul accumulated
   into the score PSUM, only for mask blocks that are not all-zero. Blocks
   whose mask is <= -1e8 everywhere (causal upper triangle) are skipped
   entirely (exp underflows to exactly 0 in the reference as well).
 - attention output is produced transposed [dv, q]; wo consumes it directly
   and the partial output is written [m, s]; host transposes once.
"""

import os
import sys

sys.path.insert(0, "/opt/trn_rl_repo")

import numpy as np
import ml_dtypes

import concourse.bass as bass
import concourse.tile as tile
import concourse.mybir as mybir
from concourse import bacc
from concourse.bass_utils import run_bass_kernel_spmd
from concourse.masks import make_identity

BF16 = mybir.dt.bfloat16
F32 = mybir.dt.float32
NPBF16 = ml_dtypes.bfloat16

S = 4096          # sequence length
D = 2048          # model dim
H = 16            # total heads
HPC = 2           # heads per core
NCORES = 8
L = 1024          # kv lora rank
ROPE = 64
NOPE = 64
VH = 128          # v head dim
SCALE = 128.0 ** -0.5
EPS = 1e-6

SB = 512          # free-dim block size
NSB = S // SB     # 8
NE = D // 128     # 16 e-chunks
NL = L // 128     # 8 latent chunks
NKT = S // 128    # 32 k tiles

USE_AG = os.environ.get("KERNEL_NO_AG", "0") != "1"

last_results = None   # BassKernelResults of the most recent run (for test.py)

_BUILD_CACHE: dict = {}


def _bcast128(ap, n):
    """[1, n] DRAM AP -> [128, n] stride-0 partition broadcast AP."""
    return bass.AP(tensor=ap.tensor, offset=ap.offset, ap=[[0, 128], [1, n]])


def _build(skip, add, use_ag):
    """Build + schedule the per-core Bass program.

    skip/add: [NKT][NSB] bool grids over (k-tile, q-block) mask blocks.
    """
    nc = bacc.Bacc("TRN2", target_bir_lowering=False, debug=False,
                   num_devices=NCORES)

    need_mask = bool(np.asarray(add).any())

    xT_d = nc.dram_tensor("xT", [D, S], BF16, kind="ExternalInput")
    cosT_d = nc.dram_tensor("cosT", [32, S], F32, kind="ExternalInput")
    sinT_d = nc.dram_tensor("sinT", [32, S], F32, kind="ExternalInput")
    wqT_d = nc.dram_tensor("wqT", [D, 128 * HPC], BF16, kind="ExternalInput")
    wkvaT_d = nc.dram_tensor("wkvaT", [D, L + ROPE], BF16, kind="ExternalInput")
    wkvbk_d = nc.dram_tensor("wkvbTk", [L, NOPE * HPC], BF16, kind="ExternalInput")
    wkvbv_d = nc.dram_tensor("wkvbTv", [L, VH * HPC], BF16, kind="ExternalInput")
    woT_d = nc.dram_tensor("woT", [VH * HPC, D], BF16, kind="ExternalInput")
    if use_ag:
        xTs_d = nc.dram_tensor("xTs", [D, SB], BF16, kind="ExternalInput")
        cosS_d = nc.dram_tensor("cosS", [32, SB], F32, kind="ExternalInput")
        sinS_d = nc.dram_tensor("sinS", [32, SB], F32, kind="ExternalInput")
    if need_mask:
        maskT_d = nc.dram_tensor("maskT", [S, S], BF16, kind="ExternalInput")
    out_d = nc.dram_tensor("out", [D, S], F32, kind="ExternalOutput")

    xT_r = xT_d[:].rearrange("(eo p) s -> p eo s", p=128)

    with tile.TileContext(nc) as tc:
        with (
            tc.tile_pool(name="singles", bufs=1) as singles,
            tc.tile_pool(name="persist", bufs=1) as persist,
            tc.tile_pool(name="drA", bufs=1, space="DRAM") as drA,
        ):
            wq_s = singles.tile([128, NE, 128 * HPC], BF16)
            nc.sync.dma_start(wq_s[:], wqT_d[:].rearrange("(eo p) c -> p eo c", p=128))
            wkva_s = singles.tile([128, NE, L + ROPE], BF16)
            nc.sync.dma_start(wkva_s[:], wkvaT_d[:].rearrange("(eo p) c -> p eo c", p=128))
            wkvbk_s = singles.tile([128, NL, NOPE * HPC], BF16)
            nc.sync.dma_start(wkvbk_s[:], wkvbk_d[:].rearrange("(lo p) c -> p lo c", p=128))
            wkvbv_s = singles.tile([128, NL, VH * HPC], BF16)
            nc.sync.dma_start(wkvbv_s[:], wkvbv_d[:].rearrange("(lo p) c -> p lo c", p=128))
            wo_s = singles.tile([128, HPC, D], BF16)
            nc.sync.dma_start(wo_s[:], woT_d[:].rearrange("(co p) m -> p co m", p=128))
            ident = singles.tile([128, 128], BF16)
            make_identity(nc, ident[:])
            ones_c = singles.tile([128, 1], BF16)
            nc.vector.memset(ones_c[:], 1.0)
            eps_t = singles.tile([1, 1], F32)
            nc.vector.memset(eps_t[:], EPS)

            q_all = persist.tile([128, HPC, S], BF16)   # per head: [pe_lo;pe_hi;nope]
            k0 = persist.tile([128, S], BF16)
            k1 = persist.tile([128, S], BF16)
            v_sb = persist.tile([128, NKT, VH * HPC], BF16)  # s-major v
            attn_T = persist.tile([128, HPC, S], BF16)  # [dv, s] per head

            def mk_rope(rpool):
                def rope(ps_pe, cos_t, sin_t, out_lo, out_hi):
                    m1 = rpool.tile([32, SB], F32, tag="m1")
                    m2 = rpool.tile([32, SB], F32, tag="m2")
                    m3 = rpool.tile([32, SB], F32, tag="m3")
                    m4 = rpool.tile([32, SB], F32, tag="m4")
                    nc.vector.tensor_mul(m1[:], ps_pe[0:32, :], cos_t[:])
                    nc.vector.tensor_mul(m2[:], ps_pe[32:64, :], sin_t[:])
                    nc.vector.tensor_mul(m3[:], ps_pe[0:32, :], sin_t[:])
                    nc.vector.tensor_mul(m4[:], ps_pe[32:64, :], cos_t[:])
                    nc.vector.tensor_sub(out_lo, m1[:], m2[:])
                    nc.vector.tensor_add(out_hi, m3[:], m4[:])
                return rope

            # -------- phase A0 (use_ag): own-shard kv_a + norm + gather ------
            if use_ag:
                lat_sh_d = drA.tile([L + ROPE, SB], BF16, tag="latsh")
                lat_full_d = drA.tile([NCORES, L + ROPE, SB], BF16, tag="latfull")
                with (
                    tc.tile_pool(name="shx", bufs=1) as shx,
                    tc.tile_pool(name="shw", bufs=1) as shw,
                    tc.tile_pool(name="shr", bufs=2) as shr,
                    tc.tile_pool(name="psS", bufs=4, space="PSUM") as psS,
                ):
                    rope_s = mk_rope(shr)
                    xs_t = shx.tile([128, NE, SB], BF16, tag="xs")
                    nc.sync.dma_start(
                        xs_t[:], xTs_d[:].rearrange("(eo p) s -> p eo s", p=128))
                    cos_s = shw.tile([32, SB], F32, tag="coss")
                    nc.sync.dma_start(cos_s[:], cosS_d[:])
                    sin_s = shw.tile([32, SB], F32, tag="sins")
                    nc.sync.dma_start(sin_s[:], sinS_d[:])

                    sq_t = shx.tile([128, NL, SB], BF16, tag="sq")
                    lat_t = shx.tile([128, NL, SB], BF16, tag="lat")
                    for lt in range(NL):
                        lp = psS.tile([128, SB], F32, tag="ps")
                        for e in range(NE):
                            nc.tensor.matmul(lp[:],
                                             wkva_s[:, e, lt * 128:(lt + 1) * 128],
                                             xs_t[:, e, :],
                                             start=(e == 0), stop=(e == NE - 1))
                        nc.scalar.activation(sq_t[:, lt, :], lp[:],
                                             mybir.ActivationFunctionType.Square)
                        nc.vector.tensor_copy(lat_t[:, lt, :], lp[:])
                    kp = psS.tile([64, SB], F32, tag="ps")
                    for e in range(NE):
                        nc.tensor.matmul(kp[:], wkva_s[:, e, L:L + ROPE],
                                         xs_t[:, e, :],
                                         start=(e == 0), stop=(e == NE - 1))
                    kpe_t = shw.tile([64, SB], BF16, tag="kpe")
                    rope_s(kp, cos_s, sin_s, kpe_t[0:32, :], kpe_t[32:64, :])

                    sp = psS.tile([1, SB], F32, tag="ps")
                    for lt in range(NL):
                        nc.tensor.matmul(sp[:], ones_c[:], sq_t[:, lt, :],
                                         start=(lt == 0), stop=(lt == NL - 1))
                    rs_t = shw.tile([1, SB], F32, tag="rs")
                    nc.scalar.activation(rs_t[:], sp[:],
                                         mybir.ActivationFunctionType.Sqrt,
                                         bias=eps_t[:], scale=1.0 / L)
                    g_t = shw.tile([1, SB], F32, tag="g")
                    nc.vector.reciprocal(g_t[:], rs_t[:])
                    g_dram = drA.tile([1, SB], F32, tag="gd")
                    nc.sync.dma_start(g_dram[:], g_t[:])
                    gb_t = shw.tile([128, SB], F32, tag="gb")
                    nc.sync.dma_start(gb_t[:], _bcast128(g_dram[:], SB))
                    for lt in range(NL):
                        nc.vector.tensor_mul(lat_t[:, lt, :], lat_t[:, lt, :],
                                             gb_t[:])
                    nc.sync.dma_start(
                        lat_sh_d[0:L, :].rearrange("(lt p) s -> p lt s", p=128),
                        lat_t[:])
                    nc.sync.dma_start(lat_sh_d[L:L + ROPE, :], kpe_t[:])
                    nc.gpsimd.collective_compute(
                        "AllGather", mybir.AluOpType.bypass,
                        replica_groups=[list(range(NCORES))],
                        ins=[lat_sh_d[:]],
                        outs=[lat_full_d[:]],
                    )

            # -------- phase A1+B: q projection + kv_b per s-block ------------
            with (
                tc.tile_pool(name="abx", bufs=2) as abx,
                tc.tile_pool(name="abw", bufs=2) as abw,
                tc.tile_pool(name="ab1", bufs=2 if use_ag else 1) as ab1,
                tc.tile_pool(name="rp", bufs=2) as rp,
                tc.tile_pool(name="psA", bufs=4, space="PSUM") as psA,
            ):
                rope = mk_rope(rp)
                for sb in range(NSB):
                    ssl = slice(sb * SB, (sb + 1) * SB)
                    x_t = abx.tile([128, NE, SB], BF16, tag="x")
                    nc.sync.dma_start(x_t[:], xT_r[:, :, ssl])
                    cos_t = abw.tile([32, SB], F32, tag="cos")
                    nc.sync.dma_start(cos_t[:], cosT_d[:, ssl])
                    sin_t = abw.tile([32, SB], F32, tag="sin")
                    nc.sync.dma_start(sin_t[:], sinT_d[:, ssl])

                    # q projection (2 head-tiles), rope applied from PSUM
                    for ct in range(HPC):
                        qp = psA.tile([128, SB], F32, tag="ps")
                        for e in range(NE):
                            nc.tensor.matmul(qp[:], wq_s[:, e, ct * 128:(ct + 1) * 128],
                                             x_t[:, e, :], start=(e == 0), stop=(e == NE - 1))
                        nc.scalar.copy(q_all[64:128, ct, ssl], qp[64:128, :])
                        rope(qp, cos_t, sin_t,
                             q_all[0:32, ct, ssl], q_all[32:64, ct, ssl])

                    if use_ag:
                        # normalized latent + roped k_pe from the AllGather
                        lg_t = ab1.tile([128, NL, SB], BF16, tag="lat")
                        nc.sync.dma_start(
                            lg_t[:],
                            lat_full_d[sb, 0:L, :].rearrange("(lt p) s -> p lt s", p=128))
                        nc.sync.dma_start(k0[0:64, ssl], lat_full_d[sb, L:L + ROPE, :])
                        nc.vector.tensor_copy(k1[0:64, ssl], k0[0:64, ssl])

                        kbp = psA.tile([128, SB], F32, tag="ps")
                        for lt in range(NL):
                            nc.tensor.matmul(kbp[:], wkvbk_s[:, lt, :], lg_t[:, lt, :],
                                             start=(lt == 0), stop=(lt == NL - 1))
                        nc.vector.tensor_copy(k0[64:128, ssl], kbp[0:64, :])
                        nc.scalar.copy(k1[64:128, ssl], kbp[64:128, :])

                        for st in range(SB // 128):
                            vp = psA.tile([128, VH * HPC], F32, tag="ps")
                            for lt in range(NL):
                                nc.tensor.matmul(vp[:],
                                                 lg_t[:, lt, st * 128:(st + 1) * 128],
                                                 wkvbv_s[:, lt, :],
                                                 start=(lt == 0), stop=(lt == NL - 1))
                            nc.scalar.copy(v_sb[:, sb * 4 + st, :], vp[:])
                    else:
                        # replicated kv_a path (no collectives)
                        sq_t = ab1.tile([128, NL, SB], BF16, tag="sq")
                        lat_t = ab1.tile([128, NL, SB], BF16, tag="lat")
                        for lt in range(NL):
                            lp = psA.tile([128, SB], F32, tag="ps")
                            for e in range(NE):
                                nc.tensor.matmul(lp[:], wkva_s[:, e, lt * 128:(lt + 1) * 128],
                                                 x_t[:, e, :], start=(e == 0), stop=(e == NE - 1))
                            nc.scalar.activation(sq_t[:, lt, :], lp[:],
                                                 mybir.ActivationFunctionType.Square)
                            nc.vector.tensor_copy(lat_t[:, lt, :], lp[:])

                        kp = psA.tile([64, SB], F32, tag="ps")
                        for e in range(NE):
                            nc.tensor.matmul(kp[:], wkva_s[:, e, L:L + ROPE],
                                             x_t[:, e, :], start=(e == 0), stop=(e == NE - 1))
                        rope(kp, cos_t, sin_t, k0[0:32, ssl], k0[32:64, ssl])
                        nc.vector.tensor_copy(k1[0:64, ssl], k0[0:64, ssl])

                        # rmsnorm scale g; applied to kv_b outputs so the kv_b
                        # matmuls don't wait on this chain
                        sp = psA.tile([1, SB], F32, tag="ps")
                        for lt in range(NL):
                            nc.tensor.matmul(sp[:], ones_c[:], sq_t[:, lt, :],
                                             start=(lt == 0), stop=(lt == NL - 1))
                        rs_t = abw.tile([1, SB], F32, tag="rs")
                        nc.scalar.activation(rs_t[:], sp[:], mybir.ActivationFunctionType.Sqrt,
                                             bias=eps_t[:], scale=1.0 / L)
                        g_t = abw.tile([1, SB], F32, tag="g")
                        nc.vector.reciprocal(g_t[:], rs_t[:])
                        g_dram = drA.tile([1, SB], F32, tag="gd")
                        nc.sync.dma_start(g_dram[:], g_t[:])
                        gb_t = abw.tile([128, SB], F32, tag="gb")
                        nc.sync.dma_start(gb_t[:], _bcast128(g_dram[:], SB))
                        gc_t = abw.tile([128, SB // 128], F32, tag="gc")
                        for st in range(SB // 128):
                            nc.sync.dma_start(
                                gc_t[:, st:st + 1],
                                g_dram[0:1, st * 128:(st + 1) * 128].rearrange("o p -> p o"))

                        kbp = psA.tile([128, SB], F32, tag="ps")
                        for lt in range(NL):
                            nc.tensor.matmul(kbp[:], wkvbk_s[:, lt, :], lat_t[:, lt, :],
                                             start=(lt == 0), stop=(lt == NL - 1))
                        nc.vector.tensor_mul(k0[64:128, ssl], kbp[0:64, :], gb_t[64:128, :])
                        nc.vector.tensor_mul(k1[64:128, ssl], kbp[64:128, :], gb_t[64:128, :])

                        for st in range(SB // 128):
                            vp = psA.tile([128, VH * HPC], F32, tag="ps")
                            for lt in range(NL):
                                nc.tensor.matmul(vp[:], lat_t[:, lt, st * 128:(st + 1) * 128],
                                                 wkvbv_s[:, lt, :],
                                                 start=(lt == 0), stop=(lt == NL - 1))
                            nc.vector.tensor_scalar_mul(v_sb[:, sb * 4 + st, :], vp[:],
                                                        gc_t[:, st:st + 1])

            # ---------------- phase C: attention -----------------
            with (
                tc.tile_pool(name="mp", bufs=3) as mp,
                tc.tile_pool(name="pb", bufs=2) as pb,
                tc.tile_pool(name="dvp", bufs=2) as dvp,
                tc.tile_pool(name="ost", bufs=3) as ost,
                tc.tile_pool(name="psatt", bufs=4, space="PSUM") as psatt,
                tc.tile_pool(name="psden", bufs=2, space="PSUM") as psden,
                tc.tile_pool(name="pso", bufs=2, space="PSUM") as pso,
                tc.tile_pool(name="drC", bufs=2, space="DRAM") as drC,
            ):
                for qb in range(NSB):
                    for h in range(HPC):
                        kh = k0 if h == 0 else k1
                        qsl = slice(qb * SB, (qb + 1) * SB)
                        active = [ki for ki in range(NKT) if not skip[ki][qb]]
                        probsT = pb.tile([128, NKT, SB], BF16, tag="probsT")
                        for ki in active:
                            ap_ = psatt.tile([128, SB], F32, tag="att")
                            has_mask = add[ki][qb]
                            nc.tensor.matmul(ap_[:], kh[:, ki * 128:(ki + 1) * 128],
                                             q_all[:, h, qsl],
                                             start=True, stop=not has_mask)
                            if has_mask:
                                m_t = mp.tile([128, SB], BF16, tag="mask")
                                nc.sync.dma_start(
                                    m_t[:], maskT_d[ki * 128:(ki + 1) * 128, qsl])
                                nc.tensor.matmul(ap_[:], ident[:], m_t[:],
                                                 start=False, stop=True)
                            nc.scalar.activation(probsT[:, ki, :], ap_[:],
                                                 mybir.ActivationFunctionType.Exp,
                                                 scale=SCALE)
                        dp = psden.tile([1, SB], F32, tag="den")
                        for i, ki in enumerate(active):
                            nc.tensor.matmul(dp[:], ones_c[:], probsT[:, ki, :],
                                             start=(i == 0), stop=(i == len(active) - 1))
                        dinv = dvp.tile([1, SB], F32, tag="dinv")
                        nc.vector.reciprocal(dinv[:], dp[:])
                        dv_dram = drC.tile([1, SB], F32, tag="dvd")
                        nc.sync.dma_start(dv_dram[:], dinv[:])
                        dinvb = dvp.tile([128, SB], F32, tag="dinvb")
                        nc.sync.dma_start(dinvb[:], _bcast128(dv_dram[:], SB))
                        op_ = pso.tile([128, SB], F32, tag="o")
                        for i, ki in enumerate(active):
                            nc.tensor.matmul(op_[:], v_sb[:, ki, h * VH:(h + 1) * VH],
                                             probsT[:, ki, :],
                                             start=(i == 0), stop=(i == len(active) - 1))
                        nc.vector.tensor_mul(attn_T[:, h, qsl], op_[:], dinvb[:])

                # ---------------- phase D: output projection -----------------
                # s2 outer so each s-block's wo work is ready as soon as its
                # attention blocks finish -> overlaps the attention tail.
                for s2 in range(NSB):
                    for mt in range(D // 128):
                        wp = psatt.tile([128, SB], F32, tag="att")
                        for cc in range(HPC):
                            nc.tensor.matmul(wp[:], wo_s[:, cc, mt * 128:(mt + 1) * 128],
                                             attn_T[:, cc, s2 * SB:(s2 + 1) * SB],
                                             start=(cc == 0), stop=(cc == HPC - 1))
                        o_t = ost.tile([128, SB], F32, tag="ostage")
                        nc.any.tensor_copy(out=o_t[:], in_=wp[:])
                        nc.sync.dma_start(
                            out_d[mt * 128:(mt + 1) * 128, s2 * SB:(s2 + 1) * SB],
                            o_t[:])

    nc.compile()
    return nc, need_mask


def kernel(x, cos, sin, mask, wq, wkv_a, kv_norm_w, wkv_b, wo, start_pos=0):
    x = np.asarray(x, np.float32)
    cos = np.asarray(cos, np.float32)
    sin = np.asarray(sin, np.float32)
    mask = np.asarray(mask, np.float32)
    wq = np.asarray(wq, np.float32)
    wkv_a = np.asarray(wkv_a, np.float32)
    kv_norm_w = np.asarray(kv_norm_w, np.float32)
    wkv_b = np.asarray(wkv_b, np.float32)
    wo = np.asarray(wo, np.float32)

    # mask block metadata: [qb, qi, kt, kj]
    mr = mask.reshape(NSB, SB, NKT, 128)
    skip_qk = (mr <= -1e8).all(axis=(1, 3))          # [qb, kt]
    nonzero_qk = (mr != 0).any(axis=(1, 3))          # [qb, kt]
    skip = skip_qk.T.copy()                          # [kt, qb]
    add = (nonzero_qk & ~skip_qk).T.copy()
    key = (bool(USE_AG), skip.tobytes(), add.tobytes())
    if key not in _BUILD_CACHE:
        _BUILD_CACHE[key] = _build(skip, add, USE_AG)
    nc, need_mask = _BUILD_CACHE[key]

    # ---- host-side shard prep ----
    deint = np.concatenate([np.arange(0, ROPE, 2), np.arange(1, ROPE, 2)])
    wq_h = wq.reshape(H, 128, D)
    # per-head row order [rope deinterleaved; nope]
    qrows = np.concatenate([wq_h[:, NOPE + deint, :], wq_h[:, 0:NOPE, :]], axis=1)
    wkva_perm = np.concatenate([wkv_a[0:L], wkv_a[L + deint]], axis=0)
    wkvb_h = wkv_b.reshape(H, NOPE + VH, L)

    xT = np.ascontiguousarray(x[0].T).astype(NPBF16)
    cosT = np.ascontiguousarray(cos.T)
    sinT = np.ascontiguousarray(sin.T)
    wkvaT = np.ascontiguousarray(wkva_perm.T).astype(NPBF16)
    shared = {"xT": xT, "cosT": cosT, "sinT": sinT, "wkvaT": wkvaT}
    if need_mask:
        shared["maskT"] = np.ascontiguousarray(mask.T * (1.0 / SCALE)).astype(NPBF16)

    in_maps = []
    for c in range(NCORES):
        hs = [HPC * c + i for i in range(HPC)]
        wqT_c = np.ascontiguousarray(
            qrows[hs].reshape(128 * HPC, D).T).astype(NPBF16)
        k_rows = (wkvb_h[hs, 0:NOPE, :] * kv_norm_w[None, None, :]).reshape(
            NOPE * HPC, L)
        wkvbTk_c = np.ascontiguousarray(k_rows.T).astype(NPBF16)
        v_rows = wkvb_h[hs, NOPE:, :].reshape(VH * HPC, L)
        wkvbTv_c = np.ascontiguousarray(v_rows.T).astype(NPBF16)
        woT_c = np.ascontiguousarray(
            wo[:, hs[0] * VH:(hs[-1] + 1) * VH].T).astype(NPBF16)
        m = dict(shared)
        m.update({"wqT": wqT_c, "wkvbTk": wkvbTk_c, "wkvbTv": wkvbTv_c,
                  "woT": woT_c})
        if USE_AG:
            ssl = slice(c * SB, (c + 1) * SB)
            m["xTs"] = np.ascontiguousarray(xT[:, ssl])
            m["cosS"] = np.ascontiguousarray(cosT[:, ssl])
            m["sinS"] = np.ascontiguousarray(sinT[:, ssl])
        in_maps.append(m)

    trace = os.environ.get("KERNEL_TRACE", "0") == "1"
    if trace:
        _install_ntff_hook()
    global last_results
    last_results = run_bass_kernel_spmd(nc, in_maps, core_ids=list(range(NCORES)),
                                        trace=trace)
    total = np.zeros((D, S), np.float32)
    for r in last_results.results:
        total += r["out"]
    return np.ascontiguousarray(total.T)[None]


def _install_ntff_hook():
    """Register the axon NTFF profiling hook (used when KERNEL_TRACE=1)."""
    import types
    import ctypes
    import contextlib

    if "antenv.axon_hooks" in sys.modules:
        return
    try:
        so = ctypes.CDLL("/opt/axon/libaxon_pjrt.so")
        so.axon_start_nrt_profile
    except (OSError, AttributeError):
        return
    so.axon_start_nrt_profile.argtypes = [ctypes.POINTER(ctypes.c_int64),
                                          ctypes.c_size_t]
    so.axon_start_nrt_profile.restype = ctypes.c_int64
    so.axon_stop_nrt_profile.argtypes = [ctypes.c_char_p]
    so.axon_stop_nrt_profile.restype = ctypes.c_int64

    @contextlib.contextmanager
    def _hook(output_dir, device_ids):
        import jax
        jax.devices()
        if device_ids:
            ids = (ctypes.c_int64 * len(device_ids))(*device_ids)
            rc = so.axon_start_nrt_profile(ids, len(device_ids))
        else:
            rc = so.axon_start_nrt_profile(None, 0)
        if rc != 0:
            raise RuntimeError(f"axon_start_nrt_profile rc={rc}")
        try:
            yield
        finally:
            n = so.axon_stop_nrt_profile(str(output_dir).encode())
            if n < 0:
                raise RuntimeError(f"axon_stop_nrt_profile rc={n}")

    mod = types.ModuleType("antenv.axon_hooks")
    mod.get_axon_ntff_profile_hook = lambda: _hook
    mod.set_axon_ntff_profile_hook = lambda h: None
    sys.modules["antenv.axon_hooks"] = mod


# revision 17
# speedup vs baseline: 1.3264x; 1.1212x over previous
"""MLA (multi-head latent attention) prefill kernel for 8 TRN2 NeuronCores.

Sharding: tensor-parallel over heads (16 heads -> 2 per core). wq / wkv_b /
wo are sliced per head on the host. The kv_a latent projection is sharded
over the sequence (each core computes 512 positions), rms-normalized and
rope'd locally, then AllGathered on-device so every core holds the full
normalized latent + k_pe. Each core computes a partial output projection
(its heads' contribution through wo); the host sums the 8 partials.

Device layout notes (all matmuls bf16, fp32 PSUM accumulation):
 - x is transposed on the host to xT [2048, 4096] so every projection runs
   channel-major: out[c, s] with weights as the stationary operand.
 - Per-head qk channel order is [rope_lo(32); rope_hi(32); nope(64)] with the
   rope pairs deinterleaved on the host (wq / wkv_a rows permuted). RoPE then
   only combines partition ranges [0:32] x [32:64] straight out of PSUM,
   which the hardware allows (PSUM operands are exempt from the equal-base-
   partition rule).
 - Scores are computed k-major: st[k, q] = (k_tile)^T q. Softmax sums over k
   (partitions) via a ones-column matmul; exp runs on the ACT engine reading
   PSUM directly, with the 1/sqrt(d) scale folded in, writing bf16 probs.
 - The additive mask is folded in with an identity-weight matmul accumulated
   into the score PSUM, only for mask blocks that are not all-zero. Blocks
   whose mask is <= -1e8 everywhere (causal upper triangle) are skipped
   entirely (exp underflows to exactly 0 in the reference as well).
 - attention output is produced transposed [dv, q]; wo consumes it directly
   and the partial output is written [m, s]; host transposes once.
"""

import os
import sys

sys.path.insert(0, "/opt/trn_rl_repo")

import numpy as np
import ml_dtypes

import concourse.bass as bass
import concourse.tile as tile
import concourse.mybir as mybir
from concourse import bacc
from concourse.bass_utils import run_bass_kernel_spmd
from concourse.masks import make_identity

BF16 = mybir.dt.bfloat16
F32 = mybir.dt.float32
NPBF16 = ml_dtypes.bfloat16

S = 4096          # sequence length
D = 2048          # model dim
H = 16            # total heads
HPC = 2           # heads per core
NCORES = 8
L = 1024          # kv lora rank
ROPE = 64
NOPE = 64
VH = 128          # v head dim
SCALE = 128.0 ** -0.5
EPS = 1e-6

SB = 512          # free-dim block size
NSB = S // SB     # 8
NE = D // 128     # 16 e-chunks
NL = L // 128     # 8 latent chunks
NKT = S // 128    # 32 k tiles

USE_AG = os.environ.get("KERNEL_NO_AG", "0") != "1"

last_results = None   # BassKernelResults of the most recent run (for test.py)

_BUILD_CACHE: dict = {}


def _bcast128(ap, n):
    """[1, n] DRAM AP -> [128, n] stride-0 partition broadcast AP."""
    return bass.AP(tensor=ap.tensor, offset=ap.offset, ap=[[0, 128], [1, n]])


def _build(skip, add, use_ag):
    """Build + schedule the per-core Bass program.

    skip/add: [NKT][NSB] bool grids over (k-tile, q-block) mask blocks.
    """
    nc = bacc.Bacc("TRN2", target_bir_lowering=False, debug=False,
                   num_devices=NCORES)

    need_mask = bool(np.asarray(add).any())

    xT_d = nc.dram_tensor("xT", [D, S], BF16, kind="ExternalInput")
    cosT_d = nc.dram_tensor("cosT", [32, S], F32, kind="ExternalInput")
    sinT_d = nc.dram_tensor("sinT", [32, S], F32, kind="ExternalInput")
    wqT_d = nc.dram_tensor("wqT", [D, 128 * HPC], BF16, kind="ExternalInput")
    wkvaT_d = nc.dram_tensor("wkvaT", [D, L + ROPE], BF16, kind="ExternalInput")
    wkvbk_d = nc.dram_tensor("wkvbTk", [L, NOPE * HPC], BF16, kind="ExternalInput")
    wkvbv_d = nc.dram_tensor("wkvbTv", [L, VH * HPC], BF16, kind="ExternalInput")
    woT_d = nc.dram_tensor("woT", [VH * HPC, D], BF16, kind="ExternalInput")
    if use_ag:
        xTs_d = nc.dram_tensor("xTs", [D, SB], BF16, kind="ExternalInput")
        cosS_d = nc.dram_tensor("cosS", [32, SB], F32, kind="ExternalInput")
        sinS_d = nc.dram_tensor("sinS", [32, SB], F32, kind="ExternalInput")
    if need_mask:
        maskT_d = nc.dram_tensor("maskT", [S, S], BF16, kind="ExternalInput")
    out_d = nc.dram_tensor("out", [D, S], F32, kind="ExternalOutput")

    xT_r = xT_d[:].rearrange("(eo p) s -> p eo s", p=128)

    with tile.TileContext(nc) as tc:
        with (
            tc.tile_pool(name="singles", bufs=1) as singles,
            tc.tile_pool(name="persist", bufs=1) as persist,
            tc.tile_pool(name="drA", bufs=1, space="DRAM") as drA,
        ):
            wq_s = singles.tile([128, NE, 128 * HPC], BF16)
            nc.sync.dma_start(wq_s[:], wqT_d[:].rearrange("(eo p) c -> p eo c", p=128))
            # split per e-chunk so the first kv_a matmuls start early
            wkva_s = singles.tile([128, NE, L + ROPE], BF16)
            wkva_r = wkvaT_d[:].rearrange("(eo p) c -> p eo c", p=128)
            for e in range(NE):
                nc.sync.dma_start(wkva_s[:, e, :], wkva_r[:, e, :])
            wkvbk_s = singles.tile([128, NL, NOPE * HPC], BF16)
            nc.sync.dma_start(wkvbk_s[:], wkvbk_d[:].rearrange("(lo p) c -> p lo c", p=128))
            wkvbv_s = singles.tile([128, NL, VH * HPC], BF16)
            nc.sync.dma_start(wkvbv_s[:], wkvbv_d[:].rearrange("(lo p) c -> p lo c", p=128))
            wo_s = singles.tile([128, HPC, D], BF16)
            nc.sync.dma_start(wo_s[:], woT_d[:].rearrange("(co p) m -> p co m", p=128))
            ident = singles.tile([128, 128], BF16)
            make_identity(nc, ident[:])
            ones_c = singles.tile([128, 1], BF16)
            nc.vector.memset(ones_c[:], 1.0)
            eps_t = singles.tile([1, 1], F32)
            nc.vector.memset(eps_t[:], EPS)

            q_all = persist.tile([128, HPC, S], BF16)   # per head: [pe_lo;pe_hi;nope]
            k0 = persist.tile([128, S], BF16)
            k1 = persist.tile([128, S], BF16)
            v_sb = persist.tile([128, NKT, VH * HPC], BF16)  # s-major v
            attn_T = persist.tile([128, HPC, S], BF16)  # [dv, s] per head

            def mk_rope(rpool):
                def rope(ps_pe, cos_t, sin_t, out_lo, out_hi):
                    m1 = rpool.tile([32, SB], F32, tag="m1")
                    m2 = rpool.tile([32, SB], F32, tag="m2")
                    m3 = rpool.tile([32, SB], F32, tag="m3")
                    m4 = rpool.tile([32, SB], F32, tag="m4")
                    nc.vector.tensor_mul(m1[:], ps_pe[0:32, :], cos_t[:])
                    nc.vector.tensor_mul(m2[:], ps_pe[32:64, :], sin_t[:])
                    nc.vector.tensor_mul(m3[:], ps_pe[0:32, :], sin_t[:])
                    nc.vector.tensor_mul(m4[:], ps_pe[32:64, :], cos_t[:])
                    nc.vector.tensor_sub(out_lo, m1[:], m2[:])
                    nc.vector.tensor_add(out_hi, m3[:], m4[:])
                return rope

            # -------- phase A0 (use_ag): own-shard kv_a + norm + gather ------
            if use_ag:
                lat_sh_d = drA.tile([L + ROPE, SB], BF16, tag="latsh")
                lat_full_d = drA.tile([NCORES, L + ROPE, SB], BF16, tag="latfull")
                with (
                    tc.tile_pool(name="shx", bufs=1) as shx,
                    tc.tile_pool(name="shw", bufs=1) as shw,
                    tc.tile_pool(name="shr", bufs=2) as shr,
                    tc.tile_pool(name="psS", bufs=4, space="PSUM") as psS,
                ):
                    rope_s = mk_rope(shr)
                    xs_t = shx.tile([128, NE, SB], BF16, tag="xs")
                    xs_r = xTs_d[:].rearrange("(eo p) s -> p eo s", p=128)
                    for e in range(NE):
                        nc.sync.dma_start(xs_t[:, e, :], xs_r[:, e, :])
                    cos_s = shw.tile([32, SB], F32, tag="coss")
                    nc.sync.dma_start(cos_s[:], cosS_d[:])
                    sin_s = shw.tile([32, SB], F32, tag="sins")
                    nc.sync.dma_start(sin_s[:], sinS_d[:])

                    sq_t = shx.tile([128, NL, SB], BF16, tag="sq")
                    lat_t = shx.tile([128, NL, SB], BF16, tag="lat")
                    for lt in range(NL):
                        lp = psS.tile([128, SB], F32, tag="ps")
                        for e in range(NE):
                            nc.tensor.matmul(lp[:],
                                             wkva_s[:, e, lt * 128:(lt + 1) * 128],
                                             xs_t[:, e, :],
                                             start=(e == 0), stop=(e == NE - 1))
                        nc.scalar.activation(sq_t[:, lt, :], lp[:],
                                             mybir.ActivationFunctionType.Square)
                        nc.vector.tensor_copy(lat_t[:, lt, :], lp[:])
                    kp = psS.tile([64, SB], F32, tag="ps")
                    for e in range(NE):
                        nc.tensor.matmul(kp[:], wkva_s[:, e, L:L + ROPE],
                                         xs_t[:, e, :],
                                         start=(e == 0), stop=(e == NE - 1))
                    kpe_t = shw.tile([64, SB], BF16, tag="kpe")
                    rope_s(kp, cos_s, sin_s, kpe_t[0:32, :], kpe_t[32:64, :])

                    sp = psS.tile([1, SB], F32, tag="ps")
                    for lt in range(NL):
                        nc.tensor.matmul(sp[:], ones_c[:], sq_t[:, lt, :],
                                         start=(lt == 0), stop=(lt == NL - 1))
                    rs_t = shw.tile([1, SB], F32, tag="rs")
                    nc.scalar.activation(rs_t[:], sp[:],
                                         mybir.ActivationFunctionType.Sqrt,
                                         bias=eps_t[:], scale=1.0 / L)
                    g_t = shw.tile([1, SB], F32, tag="g")
                    nc.vector.reciprocal(g_t[:], rs_t[:])
                    g_dram = drA.tile([1, SB], F32, tag="gd")
                    nc.sync.dma_start(g_dram[:], g_t[:])
                    gb_t = shw.tile([128, SB], F32, tag="gb")
                    nc.sync.dma_start(gb_t[:], _bcast128(g_dram[:], SB))
                    for lt in range(NL):
                        nc.vector.tensor_mul(lat_t[:, lt, :], lat_t[:, lt, :],
                                             gb_t[:])
                    nc.sync.dma_start(
                        lat_sh_d[0:L, :].rearrange("(lt p) s -> p lt s", p=128),
                        lat_t[:])
                    nc.sync.dma_start(lat_sh_d[L:L + ROPE, :], kpe_t[:])
                    nc.gpsimd.collective_compute(
                        "AllGather", mybir.AluOpType.bypass,
                        replica_groups=[list(range(NCORES))],
                        ins=[lat_sh_d[:]],
                        outs=[lat_full_d[:]],
                    )

            # -------- phase A1+B: q projection + kv_b per s-block ------------
            with (
                tc.tile_pool(name="abx", bufs=2) as abx,
                tc.tile_pool(name="abw", bufs=2) as abw,
                tc.tile_pool(name="ab1", bufs=2 if use_ag else 1) as ab1,
                tc.tile_pool(name="rp", bufs=2) as rp,
                tc.tile_pool(name="psA", bufs=4, space="PSUM") as psA,
            ):
                rope = mk_rope(rp)

                def q_proj(sb):
                    ssl = slice(sb * SB, (sb + 1) * SB)
                    x_t = abx.tile([128, NE, SB], BF16, tag="x")
                    nc.sync.dma_start(x_t[:], xT_r[:, :, ssl])
                    cos_t = abw.tile([32, SB], F32, tag="cos")
                    nc.sync.dma_start(cos_t[:], cosT_d[:, ssl])
                    sin_t = abw.tile([32, SB], F32, tag="sin")
                    nc.sync.dma_start(sin_t[:], sinT_d[:, ssl])
                    for ct in range(HPC):
                        qp = psA.tile([128, SB], F32, tag="ps")
                        for e in range(NE):
                            nc.tensor.matmul(qp[:], wq_s[:, e, ct * 128:(ct + 1) * 128],
                                             x_t[:, e, :], start=(e == 0), stop=(e == NE - 1))
                        nc.scalar.copy(q_all[64:128, ct, ssl], qp[64:128, :])
                        rope(qp, cos_t, sin_t,
                             q_all[0:32, ct, ssl], q_all[32:64, ct, ssl])
                    return x_t, cos_t, sin_t

                if use_ag:
                    # all q projections first: they overlap the AllGather,
                    # which the kv_b loop below depends on.
                    for sb in range(NSB):
                        q_proj(sb)
                    for sb in range(NSB):
                        ssl = slice(sb * SB, (sb + 1) * SB)
                        # normalized latent + roped k_pe from the AllGather
                        lg_t = ab1.tile([128, NL, SB], BF16, tag="lat")
                        nc.sync.dma_start(
                            lg_t[:],
                            lat_full_d[sb, 0:L, :].rearrange("(lt p) s -> p lt s", p=128))
                        nc.sync.dma_start(k0[0:64, ssl], lat_full_d[sb, L:L + ROPE, :])
                        nc.vector.tensor_copy(k1[0:64, ssl], k0[0:64, ssl])

                        kbp = psA.tile([128, SB], F32, tag="ps")
                        for lt in range(NL):
                            nc.tensor.matmul(kbp[:], wkvbk_s[:, lt, :], lg_t[:, lt, :],
                                             start=(lt == 0), stop=(lt == NL - 1))
                        nc.vector.tensor_copy(k0[64:128, ssl], kbp[0:64, :])
                        nc.vector.tensor_copy(k1[64:128, ssl], kbp[64:128, :])

                        for st in range(SB // 128):
                            vp = psA.tile([128, VH * HPC], F32, tag="ps")
                            for lt in range(NL):
                                nc.tensor.matmul(vp[:],
                                                 lg_t[:, lt, st * 128:(st + 1) * 128],
                                                 wkvbv_s[:, lt, :],
                                                 start=(lt == 0), stop=(lt == NL - 1))
                            nc.scalar.copy(v_sb[:, sb * 4 + st, :], vp[:])
                else:
                    for sb in range(NSB):
                        ssl = slice(sb * SB, (sb + 1) * SB)
                        x_t, cos_t, sin_t = q_proj(sb)
                        # replicated kv_a path (no collectives)
                        sq_t = ab1.tile([128, NL, SB], BF16, tag="sq")
                        lat_t = ab1.tile([128, NL, SB], BF16, tag="lat")
                        for lt in range(NL):
                            lp = psA.tile([128, SB], F32, tag="ps")
                            for e in range(NE):
                                nc.tensor.matmul(lp[:], wkva_s[:, e, lt * 128:(lt + 1) * 128],
                                                 x_t[:, e, :], start=(e == 0), stop=(e == NE - 1))
                            nc.scalar.activation(sq_t[:, lt, :], lp[:],
                                                 mybir.ActivationFunctionType.Square)
                            nc.vector.tensor_copy(lat_t[:, lt, :], lp[:])

                        kp = psA.tile([64, SB], F32, tag="ps")
                        for e in range(NE):
                            nc.tensor.matmul(kp[:], wkva_s[:, e, L:L + ROPE],
                                             x_t[:, e, :], start=(e == 0), stop=(e == NE - 1))
                        rope(kp, cos_t, sin_t, k0[0:32, ssl], k0[32:64, ssl])
                        nc.vector.tensor_copy(k1[0:64, ssl], k0[0:64, ssl])

                        # rmsnorm scale g; applied to kv_b outputs so the kv_b
                        # matmuls don't wait on this chain
                        sp = psA.tile([1, SB], F32, tag="ps")
                        for lt in range(NL):
                            nc.tensor.matmul(sp[:], ones_c[:], sq_t[:, lt, :],
                                             start=(lt == 0), stop=(lt == NL - 1))
                        rs_t = abw.tile([1, SB], F32, tag="rs")
                        nc.scalar.activation(rs_t[:], sp[:], mybir.ActivationFunctionType.Sqrt,
                                             bias=eps_t[:], scale=1.0 / L)
                        g_t = abw.tile([1, SB], F32, tag="g")
                        nc.vector.reciprocal(g_t[:], rs_t[:])
                        g_dram = drA.tile([1, SB], F32, tag="gd")
                        nc.sync.dma_start(g_dram[:], g_t[:])
                        gb_t = abw.tile([128, SB], F32, tag="gb")
                        nc.sync.dma_start(gb_t[:], _bcast128(g_dram[:], SB))
                        gc_t = abw.tile([128, SB // 128], F32, tag="gc")
                        for st in range(SB // 128):
                            nc.sync.dma_start(
                                gc_t[:, st:st + 1],
                                g_dram[0:1, st * 128:(st + 1) * 128].rearrange("o p -> p o"))

                        kbp = psA.tile([128, SB], F32, tag="ps")
                        for lt in range(NL):
                            nc.tensor.matmul(kbp[:], wkvbk_s[:, lt, :], lat_t[:, lt, :],
                                             start=(lt == 0), stop=(lt == NL - 1))
                        nc.vector.tensor_mul(k0[64:128, ssl], kbp[0:64, :], gb_t[64:128, :])
                        nc.vector.tensor_mul(k1[64:128, ssl], kbp[64:128, :], gb_t[64:128, :])

                        for st in range(SB // 128):
                            vp = psA.tile([128, VH * HPC], F32, tag="ps")
                            for lt in range(NL):
                                nc.tensor.matmul(vp[:], lat_t[:, lt, st * 128:(st + 1) * 128],
                                                 wkvbv_s[:, lt, :],
                                                 start=(lt == 0), stop=(lt == NL - 1))
                            nc.vector.tensor_scalar_mul(v_sb[:, sb * 4 + st, :], vp[:],
                                                        gc_t[:, st:st + 1])

            # ---------------- phase C: attention -----------------
            with (
                tc.tile_pool(name="mp", bufs=3) as mp,
                tc.tile_pool(name="pb", bufs=2) as pb,
                tc.tile_pool(name="dvp", bufs=2) as dvp,
                tc.tile_pool(name="ost", bufs=3) as ost,
                tc.tile_pool(name="psatt", bufs=4, space="PSUM") as psatt,
                tc.tile_pool(name="psden", bufs=2, space="PSUM") as psden,
                tc.tile_pool(name="pso", bufs=2, space="PSUM") as pso,
                tc.tile_pool(name="drC", bufs=2, space="DRAM") as drC,
            ):
                for qb in range(NSB):
                    for h in range(HPC):
                        kh = k0 if h == 0 else k1
                        qsl = slice(qb * SB, (qb + 1) * SB)
                        active = [ki for ki in range(NKT) if not skip[ki][qb]]
                        probsT = pb.tile([128, NKT, SB], BF16, tag="probsT")
                        for ki in active:
                            ap_ = psatt.tile([128, SB], F32, tag="att")
                            has_mask = add[ki][qb]
                            nc.tensor.matmul(ap_[:], kh[:, ki * 128:(ki + 1) * 128],
                                             q_all[:, h, qsl],
                                             start=True, stop=not has_mask)
                            if has_mask:
                                m_t = mp.tile([128, SB], BF16, tag="mask")
                                nc.sync.dma_start(
                                    m_t[:], maskT_d[ki * 128:(ki + 1) * 128, qsl])
                                nc.tensor.matmul(ap_[:], ident[:], m_t[:],
                                                 start=False, stop=True)
                            nc.scalar.activation(probsT[:, ki, :], ap_[:],
                                                 mybir.ActivationFunctionType.Exp,
                                                 scale=SCALE)
                        dp = psden.tile([1, SB], F32, tag="den")
                        for i, ki in enumerate(active):
                            nc.tensor.matmul(dp[:], ones_c[:], probsT[:, ki, :],
                                             start=(i == 0), stop=(i == len(active) - 1))
                        dinv = dvp.tile([1, SB], F32, tag="dinv")
                        nc.vector.reciprocal(dinv[:], dp[:])
                        dv_dram = drC.tile([1, SB], F32, tag="dvd")
                        nc.sync.dma_start(dv_dram[:], dinv[:])
                        dinvb = dvp.tile([128, SB], F32, tag="dinvb")
                        nc.sync.dma_start(dinvb[:], _bcast128(dv_dram[:], SB))
                        op_ = pso.tile([128, SB], F32, tag="o")
                        for i, ki in enumerate(active):
                            nc.tensor.matmul(op_[:], v_sb[:, ki, h * VH:(h + 1) * VH],
                                             probsT[:, ki, :],
                                             start=(i == 0), stop=(i == len(active) - 1))
                        nc.vector.tensor_mul(attn_T[:, h, qsl], op_[:], dinvb[:])

                # ---------------- phase D: output projection -----------------
                # s2 outer so each s-block's wo work is ready as soon as its
                # attention blocks finish -> overlaps the attention tail.
                for s2 in range(NSB):
                    for mt in range(D // 128):
                        wp = psatt.tile([128, SB], F32, tag="att")
                        for cc in range(HPC):
                            nc.tensor.matmul(wp[:], wo_s[:, cc, mt * 128:(mt + 1) * 128],
                                             attn_T[:, cc, s2 * SB:(s2 + 1) * SB],
                                             start=(cc == 0), stop=(cc == HPC - 1))
                        o_t = ost.tile([128, SB], F32, tag="ostage")
                        nc.any.tensor_copy(out=o_t[:], in_=wp[:])
                        nc.sync.dma_start(
                            out_d[mt * 128:(mt + 1) * 128, s2 * SB:(s2 + 1) * SB],
                            o_t[:])

    nc.compile()
    return nc, need_mask


def kernel(x, cos, sin, mask, wq, wkv_a, kv_norm_w, wkv_b, wo, start_pos=0):
    x = np.asarray(x, np.float32)
    cos = np.asarray(cos, np.float32)
    sin = np.asarray(sin, np.float32)
    mask = np.asarray(mask, np.float32)
    wq = np.asarray(wq, np.float32)
    wkv_a = np.asarray(wkv_a, np.float32)
    kv_norm_w = np.asarray(kv_norm_w, np.float32)
    wkv_b = np.asarray(wkv_b, np.float32)
    wo = np.asarray(wo, np.float32)

    # mask block metadata: [qb, qi, kt, kj]
    mr = mask.reshape(NSB, SB, NKT, 128)
    skip_qk = (mr <= -1e8).all(axis=(1, 3))          # [qb, kt]
    nonzero_qk = (mr != 0).any(axis=(1, 3))          # [qb, kt]
    skip = skip_qk.T.copy()                          # [kt, qb]
    add = (nonzero_qk & ~skip_qk).T.copy()
    key = (bool(USE_AG), skip.tobytes(), add.tobytes())
    if key not in _BUILD_CACHE:
        _BUILD_CACHE[key] = _build(skip, add, USE_AG)
    nc, need_mask = _BUILD_CACHE[key]

    # ---- host-side shard prep ----
    deint = np.concatenate([np.arange(0, ROPE, 2), np.arange(1, ROPE, 2)])
    wq_h = wq.reshape(H, 128, D)
    # per-head row order [rope deinterleaved; nope]
    qrows = np.concatenate([wq_h[:, NOPE + deint, :], wq_h[:, 0:NOPE, :]], axis=1)
    wkva_perm = np.concatenate([wkv_a[0:L], wkv_a[L + deint]], axis=0)
    wkvb_h = wkv_b.reshape(H, NOPE + VH, L)

    xT = np.ascontiguousarray(x[0].T).astype(NPBF16)
    cosT = np.ascontiguousarray(cos.T)
    sinT = np.ascontiguousarray(sin.T)
    wkvaT = np.ascontiguousarray(wkva_perm.T).astype(NPBF16)
    shared = {"xT": xT, "cosT": cosT, "sinT": sinT, "wkvaT": wkvaT}
    if need_mask:
        shared["maskT"] = np.ascontiguousarray(mask.T * (1.0 / SCALE)).astype(NPBF16)

    in_maps = []
    for c in range(NCORES):
        hs = [HPC * c + i for i in range(HPC)]
        wqT_c = np.ascontiguousarray(
            qrows[hs].reshape(128 * HPC, D).T).astype(NPBF16)
        k_rows = (wkvb_h[hs, 0:NOPE, :] * kv_norm_w[None, None, :]).reshape(
            NOPE * HPC, L)
        wkvbTk_c = np.ascontiguousarray(k_rows.T).astype(NPBF16)
        v_rows = wkvb_h[hs, NOPE:, :].reshape(VH * HPC, L)
        wkvbTv_c = np.ascontiguousarray(v_rows.T).astype(NPBF16)
        woT_c = np.ascontiguousarray(
            wo[:, hs[0] * VH:(hs[-1] + 1) * VH].T).astype(NPBF16)
        m = dict(shared)
        m.update({"wqT": wqT_c, "wkvbTk": wkvbTk_c, "wkvbTv": wkvbTv_c,
                  "woT": woT_c})
        if USE_AG:
            ssl = slice(c * SB, (c + 1) * SB)
            m["xTs"] = np.ascontiguousarray(xT[:, ssl])
            m["cosS"] = np.ascontiguousarray(cosT[:, ssl])
            m["sinS"] = np.ascontiguousarray(sinT[:, ssl])
        in_maps.append(m)

    trace = os.environ.get("KERNEL_TRACE", "0") == "1"
    if trace:
        _install_ntff_hook()
    global last_results
    last_results = run_bass_kernel_spmd(nc, in_maps, core_ids=list(range(NCORES)),
                                        trace=trace)
    total = np.zeros((D, S), np.float32)
    for r in last_results.results:
        total += r["out"]
    return np.ascontiguousarray(total.T)[None]


def _install_ntff_hook():
    """Register the axon NTFF profiling hook (used when KERNEL_TRACE=1)."""
    import types
    import ctypes
    import contextlib

    if "antenv.axon_hooks" in sys.modules:
        return
    try:
        so = ctypes.CDLL("/opt/axon/libaxon_pjrt.so")
        so.axon_start_nrt_profile
    except (OSError, AttributeError):
        return
    so.axon_start_nrt_profile.argtypes = [ctypes.POINTER(ctypes.c_int64),
                                          ctypes.c_size_t]
    so.axon_start_nrt_profile.restype = ctypes.c_int64
    so.axon_stop_nrt_profile.argtypes = [ctypes.c_char_p]
    so.axon_stop_nrt_profile.restype = ctypes.c_int64

    @contextlib.contextmanager
    def _hook(output_dir, device_ids):
        import jax
        jax.devices()
        if device_ids:
            ids = (ctypes.c_int64 * len(device_ids))(*device_ids)
            rc = so.axon_start_nrt_profile(ids, len(device_ids))
        else:
            rc = so.axon_start_nrt_profile(None, 0)
        if rc != 0:
            raise RuntimeError(f"axon_start_nrt_profile rc={rc}")
        try:
            yield
        finally:
            n = so.axon_stop_nrt_profile(str(output_dir).encode())
            if n < 0:
                raise RuntimeError(f"axon_stop_nrt_profile rc={n}")

    mod = types.ModuleType("antenv.axon_hooks")
    mod.get_axon_ntff_profile_hook = lambda: _hook
    mod.set_axon_ntff_profile_hook = lambda h: None
    sys.modules["antenv.axon_hooks"] = mod


# revision 27
# speedup vs baseline: 1.3448x; 1.0139x over previous
"""MLA (multi-head latent attention) prefill kernel for 8 TRN2 NeuronCores.

Sharding: tensor-parallel over heads (16 heads -> 2 per core). wq / wkv_b /
wo are sliced per head on the host. The kv_a latent projection is sharded
over the sequence (each core computes 512 positions), rms-normalized and
rope'd locally, then AllGathered on-device so every core holds the full
normalized latent + k_pe. Each core computes a partial output projection
(its heads' contribution through wo); the host sums the 8 partials.

Device layout notes (all matmuls bf16, fp32 PSUM accumulation):
 - x is transposed on the host to xT [2048, 4096] so every projection runs
   channel-major: out[c, s] with weights as the stationary operand.
 - Per-head qk channel order is [rope_lo(32); rope_hi(32); nope(64)] with the
   rope pairs deinterleaved on the host (wq / wkv_a rows permuted). RoPE then
   only combines partition ranges [0:32] x [32:64] straight out of PSUM,
   which the hardware allows (PSUM operands are exempt from the equal-base-
   partition rule).
 - Scores are computed k-major: st[k, q] = (k_tile)^T q. Softmax sums over k
   (partitions) via a ones-column matmul; exp runs on the ACT engine reading
   PSUM directly, with the 1/sqrt(d) scale folded in, writing bf16 probs.
 - The additive mask is folded in with an identity-weight matmul accumulated
   into the score PSUM, only for mask blocks that are not all-zero. Blocks
   whose mask is <= -1e8 everywhere (causal upper triangle) are skipped
   entirely (exp underflows to exactly 0 in the reference as well).
 - attention output is produced transposed [dv, q]; wo consumes it directly
   and the partial output is written [m, s]; host transposes once.
"""

import os
import sys

sys.path.insert(0, "/opt/trn_rl_repo")

import numpy as np
import ml_dtypes

import concourse.bass as bass
import concourse.tile as tile
import concourse.mybir as mybir
from concourse import bacc
from concourse.bass_utils import run_bass_kernel_spmd
from concourse.masks import make_identity

BF16 = mybir.dt.bfloat16
F32 = mybir.dt.float32
NPBF16 = ml_dtypes.bfloat16

S = 4096          # sequence length
D = 2048          # model dim
H = 16            # total heads
HPC = 2           # heads per core
NCORES = 8
L = 1024          # kv lora rank
ROPE = 64
NOPE = 64
VH = 128          # v head dim
SCALE = 128.0 ** -0.5
EPS = 1e-6

SB = 512          # free-dim block size
NSB = S // SB     # 8
NE = D // 128     # 16 e-chunks
NL = L // 128     # 8 latent chunks
NKT = S // 128    # 32 k tiles

USE_AG = os.environ.get("KERNEL_NO_AG", "0") != "1"

last_results = None   # BassKernelResults of the most recent run (for test.py)

_BUILD_CACHE: dict = {}


def _bcast128(ap, n):
    """[1, n] DRAM AP -> [128, n] stride-0 partition broadcast AP."""
    return bass.AP(tensor=ap.tensor, offset=ap.offset, ap=[[0, 128], [1, n]])


def _build(skip, add, use_ag):
    """Build + schedule the per-core Bass program.

    skip/add: [NKT][NSB] bool grids over (k-tile, q-block) mask blocks.
    """
    nc = bacc.Bacc("TRN2", target_bir_lowering=False, debug=False,
                   num_devices=NCORES)

    need_mask = bool(np.asarray(add).any())

    xT_d = nc.dram_tensor("xT", [D, S], BF16, kind="ExternalInput")
    cosT_d = nc.dram_tensor("cosT", [32, S], F32, kind="ExternalInput")
    sinT_d = nc.dram_tensor("sinT", [32, S], F32, kind="ExternalInput")
    wqT_d = nc.dram_tensor("wqT", [D, 128 * HPC], BF16, kind="ExternalInput")
    wkvaT_d = nc.dram_tensor("wkvaT", [D, L + ROPE], BF16, kind="ExternalInput")
    wkvbk_d = nc.dram_tensor("wkvbTk", [L, NOPE * HPC], BF16, kind="ExternalInput")
    wkvbv_d = nc.dram_tensor("wkvbTv", [L, VH * HPC], BF16, kind="ExternalInput")
    woT_d = nc.dram_tensor("woT", [VH * HPC, D], BF16, kind="ExternalInput")
    if use_ag:
        xTs_d = nc.dram_tensor("xTs", [D, SB], BF16, kind="ExternalInput")
        cosS_d = nc.dram_tensor("cosS", [32, SB], F32, kind="ExternalInput")
        sinS_d = nc.dram_tensor("sinS", [32, SB], F32, kind="ExternalInput")
    if need_mask:
        maskT_d = nc.dram_tensor("maskT", [S, S], BF16, kind="ExternalInput")
    out_d = nc.dram_tensor("out", [D, S], F32, kind="ExternalOutput")

    xT_r = xT_d[:].rearrange("(eo p) s -> p eo s", p=128)

    with tile.TileContext(nc) as tc:
        with (
            tc.tile_pool(name="singles", bufs=1) as singles,
            tc.tile_pool(name="persist", bufs=1) as persist,
            tc.tile_pool(name="drA", bufs=1, space="DRAM") as drA,
        ):
            ident = singles.tile([128, 128], BF16)
            make_identity(nc, ident[:])
            ones_c = singles.tile([128, 1], BF16)
            nc.vector.memset(ones_c[:], 1.0)
            eps_t = singles.tile([1, 1], F32)
            nc.vector.memset(eps_t[:], EPS)
            # the remaining weights are needed later (q proj / kv_b / wo);
            # their loads are issued after the kv_a shard section below.
            wq_s = singles.tile([128, NE, 128 * HPC], BF16)
            wkvbk_s = singles.tile([128, NL, NOPE * HPC], BF16)
            wkvbv_s = singles.tile([128, NL, VH * HPC], BF16)
            wo_s = singles.tile([128, HPC, D], BF16)

            def load_late_weights():
                nc.sync.dma_start(wq_s[:], wqT_d[:].rearrange("(eo p) c -> p eo c", p=128))
                nc.sync.dma_start(wkvbk_s[:], wkvbk_d[:].rearrange("(lo p) c -> p lo c", p=128))
                nc.sync.dma_start(wkvbv_s[:], wkvbv_d[:].rearrange("(lo p) c -> p lo c", p=128))
                nc.sync.dma_start(wo_s[:], woT_d[:].rearrange("(co p) m -> p co m", p=128))

            q_all = persist.tile([128, HPC, S], BF16)   # per head: [pe_lo;pe_hi;nope]
            k0 = persist.tile([128, S], BF16)
            k1 = persist.tile([128, S], BF16)
            v_sb = persist.tile([128, NKT, VH * HPC], BF16)  # s-major v
            attn_T = persist.tile([128, HPC, S], BF16)  # [dv, s] per head

            # wkv_a weights live only through the projection phases; scoping
            # them in a separate pool frees 34KB/partition for attention.
            from contextlib import ExitStack
            _wk = ExitStack()
            wkvap = _wk.enter_context(tc.tile_pool(name="wkvap", bufs=1))
            wkva_s = wkvap.tile([128, NE, L + ROPE], BF16)
            wkva_r = wkvaT_d[:].rearrange("(eo p) c -> p eo c", p=128)
            for e in range(NE):
                nc.sync.dma_start(wkva_s[:, e, :], wkva_r[:, e, :])

            def mk_rope(rpool):
                def rope(ps_pe, cos_t, sin_t, out_lo, out_hi):
                    m1 = rpool.tile([32, SB], F32, tag="m1")
                    m2 = rpool.tile([32, SB], F32, tag="m2")
                    m3 = rpool.tile([32, SB], F32, tag="m3")
                    m4 = rpool.tile([32, SB], F32, tag="m4")
                    nc.vector.tensor_mul(m1[:], ps_pe[0:32, :], cos_t[:])
                    nc.vector.tensor_mul(m2[:], ps_pe[32:64, :], sin_t[:])
                    nc.vector.tensor_mul(m3[:], ps_pe[0:32, :], sin_t[:])
                    nc.vector.tensor_mul(m4[:], ps_pe[32:64, :], cos_t[:])
                    nc.vector.tensor_sub(out_lo, m1[:], m2[:])
                    nc.vector.tensor_add(out_hi, m3[:], m4[:])
                return rope

            # -------- phase A0 (use_ag): own-shard kv_a + norm + gather ------
            if use_ag:
                lat_sh_d = nc.dram_tensor("lat_sh", [L + ROPE, SB], BF16)
                lat_full_d = nc.dram_tensor("lat_full", [NCORES, L + ROPE, SB],
                                            BF16, addr_space="Shared")
                with (
                    tc.tile_pool(name="shx", bufs=1) as shx,
                    tc.tile_pool(name="shw", bufs=1) as shw,
                    tc.tile_pool(name="shr", bufs=2) as shr,
                    tc.tile_pool(name="psS", bufs=4, space="PSUM") as psS,
                ):
                    rope_s = mk_rope(shr)
                    xs_t = shx.tile([128, NE, SB], BF16, tag="xs")
                    xs_r = xTs_d[:].rearrange("(eo p) s -> p eo s", p=128)
                    for e in range(NE):
                        nc.sync.dma_start(xs_t[:, e, :], xs_r[:, e, :])
                    cos_s = shw.tile([32, SB], F32, tag="coss")
                    nc.sync.dma_start(cos_s[:], cosS_d[:])
                    sin_s = shw.tile([32, SB], F32, tag="sins")
                    nc.sync.dma_start(sin_s[:], sinS_d[:])

                    sq_t = shx.tile([128, NL, SB], BF16, tag="sq")
                    lat_t = shx.tile([128, NL, SB], BF16, tag="lat")
                    for lt in range(NL):
                        lp = psS.tile([128, SB], F32, tag="ps")
                        for e in range(NE):
                            nc.tensor.matmul(lp[:],
                                             wkva_s[:, e, lt * 128:(lt + 1) * 128],
                                             xs_t[:, e, :],
                                             start=(e == 0), stop=(e == NE - 1))
                        nc.scalar.activation(sq_t[:, lt, :], lp[:],
                                             mybir.ActivationFunctionType.Square)
                        nc.vector.tensor_copy(lat_t[:, lt, :], lp[:])
                    kp = psS.tile([64, SB], F32, tag="ps")
                    for e in range(NE):
                        nc.tensor.matmul(kp[:], wkva_s[:, e, L:L + ROPE],
                                         xs_t[:, e, :],
                                         start=(e == 0), stop=(e == NE - 1))
                    kpe_t = shw.tile([64, SB], BF16, tag="kpe")
                    rope_s(kp, cos_s, sin_s, kpe_t[0:32, :], kpe_t[32:64, :])

                    sp = psS.tile([1, SB], F32, tag="ps")
                    for lt in range(NL):
                        nc.tensor.matmul(sp[:], ones_c[:], sq_t[:, lt, :],
                                         start=(lt == 0), stop=(lt == NL - 1))
                    rs_t = shw.tile([1, SB], F32, tag="rs")
                    nc.scalar.activation(rs_t[:], sp[:],
                                         mybir.ActivationFunctionType.Sqrt,
                                         bias=eps_t[:], scale=1.0 / L)
                    g_t = shw.tile([1, SB], F32, tag="g")
                    nc.vector.reciprocal(g_t[:], rs_t[:])
                    g_dram = drA.tile([1, SB], F32, tag="gd")
                    nc.sync.dma_start(g_dram[:], g_t[:])
                    gb_t = shw.tile([128, SB], F32, tag="gb")
                    nc.sync.dma_start(gb_t[:], _bcast128(g_dram[:], SB))
                    for lt in range(NL):
                        nc.vector.tensor_mul(lat_t[:, lt, :], lat_t[:, lt, :],
                                             gb_t[:])
                    nc.sync.dma_start(
                        lat_sh_d[0:L, :].rearrange("(lt p) s -> p lt s", p=128),
                        lat_t[:])
                    nc.sync.dma_start(lat_sh_d[L:L + ROPE, :], kpe_t[:])
                    nc.gpsimd.collective_compute(
                        "AllGather", mybir.AluOpType.bypass,
                        replica_groups=[list(range(NCORES))],
                        ins=[lat_sh_d[:]],
                        outs=[lat_full_d[:]],
                    )

            # -------- phase A1+B: q projection + kv_b per s-block ------------
            with (
                tc.tile_pool(name="abx", bufs=2) as abx,
                tc.tile_pool(name="abw", bufs=2) as abw,
                tc.tile_pool(name="ab1", bufs=2 if use_ag else 1) as ab1,
                tc.tile_pool(name="rp", bufs=2) as rp,
                tc.tile_pool(name="psA", bufs=4, space="PSUM") as psA,
            ):
                rope = mk_rope(rp)

                def q_proj(sb):
                    ssl = slice(sb * SB, (sb + 1) * SB)
                    x_t = abx.tile([128, NE, SB], BF16, tag="x")
                    nc.sync.dma_start(x_t[:], xT_r[:, :, ssl])
                    cos_t = abw.tile([32, SB], F32, tag="cos")
                    nc.sync.dma_start(cos_t[:], cosT_d[:, ssl])
                    sin_t = abw.tile([32, SB], F32, tag="sin")
                    nc.sync.dma_start(sin_t[:], sinT_d[:, ssl])
                    for ct in range(HPC):
                        qp = psA.tile([128, SB], F32, tag="ps")
                        for e in range(NE):
                            nc.tensor.matmul(qp[:], wq_s[:, e, ct * 128:(ct + 1) * 128],
                                             x_t[:, e, :], start=(e == 0), stop=(e == NE - 1))
                        nc.scalar.copy(q_all[64:128, ct, ssl], qp[64:128, :])
                        rope(qp, cos_t, sin_t,
                             q_all[0:32, ct, ssl], q_all[32:64, ct, ssl])
                    return x_t, cos_t, sin_t

                if use_ag:
                    # all q projections here: they overlap the AllGather,
                    # which the kv_b work (in the merged phase below) needs.
                    load_late_weights()
                    for sb in range(NSB):
                        q_proj(sb)
                else:
                    load_late_weights()
                    for sb in range(NSB):
                        ssl = slice(sb * SB, (sb + 1) * SB)
                        x_t, cos_t, sin_t = q_proj(sb)
                        # replicated kv_a path (no collectives)
                        sq_t = ab1.tile([128, NL, SB], BF16, tag="sq")
                        lat_t = ab1.tile([128, NL, SB], BF16, tag="lat")
                        for lt in range(NL):
                            lp = psA.tile([128, SB], F32, tag="ps")
                            for e in range(NE):
                                nc.tensor.matmul(lp[:], wkva_s[:, e, lt * 128:(lt + 1) * 128],
                                                 x_t[:, e, :], start=(e == 0), stop=(e == NE - 1))
                            nc.scalar.activation(sq_t[:, lt, :], lp[:],
                                                 mybir.ActivationFunctionType.Square)
                            nc.vector.tensor_copy(lat_t[:, lt, :], lp[:])

                        kp = psA.tile([64, SB], F32, tag="ps")
                        for e in range(NE):
                            nc.tensor.matmul(kp[:], wkva_s[:, e, L:L + ROPE],
                                             x_t[:, e, :], start=(e == 0), stop=(e == NE - 1))
                        rope(kp, cos_t, sin_t, k0[0:32, ssl], k0[32:64, ssl])
                        nc.vector.tensor_copy(k1[0:64, ssl], k0[0:64, ssl])

                        # rmsnorm scale g; applied to kv_b outputs so the kv_b
                        # matmuls don't wait on this chain
                        sp = psA.tile([1, SB], F32, tag="ps")
                        for lt in range(NL):
                            nc.tensor.matmul(sp[:], ones_c[:], sq_t[:, lt, :],
                                             start=(lt == 0), stop=(lt == NL - 1))
                        rs_t = abw.tile([1, SB], F32, tag="rs")
                        nc.scalar.activation(rs_t[:], sp[:], mybir.ActivationFunctionType.Sqrt,
                                             bias=eps_t[:], scale=1.0 / L)
                        g_t = abw.tile([1, SB], F32, tag="g")
                        nc.vector.reciprocal(g_t[:], rs_t[:])
                        g_dram = drA.tile([1, SB], F32, tag="gd")
                        nc.sync.dma_start(g_dram[:], g_t[:])
                        gb_t = abw.tile([128, SB], F32, tag="gb")
                        nc.sync.dma_start(gb_t[:], _bcast128(g_dram[:], SB))
                        gc_t = abw.tile([128, SB // 128], F32, tag="gc")
                        for st in range(SB // 128):
                            nc.sync.dma_start(
                                gc_t[:, st:st + 1],
                                g_dram[0:1, st * 128:(st + 1) * 128].rearrange("o p -> p o"))

                        kbp = psA.tile([128, SB], F32, tag="ps")
                        for lt in range(NL):
                            nc.tensor.matmul(kbp[:], wkvbk_s[:, lt, :], lat_t[:, lt, :],
                                             start=(lt == 0), stop=(lt == NL - 1))
                        nc.vector.tensor_mul(k0[64:128, ssl], kbp[0:64, :], gb_t[64:128, :])
                        nc.vector.tensor_mul(k1[64:128, ssl], kbp[64:128, :], gb_t[64:128, :])

                        for st in range(SB // 128):
                            vp = psA.tile([128, VH * HPC], F32, tag="ps")
                            for lt in range(NL):
                                nc.tensor.matmul(vp[:], lat_t[:, lt, st * 128:(st + 1) * 128],
                                                 wkvbv_s[:, lt, :],
                                                 start=(lt == 0), stop=(lt == NL - 1))
                            nc.vector.tensor_scalar_mul(v_sb[:, sb * 4 + st, :], vp[:],
                                                        gc_t[:, st:st + 1])

            _wk.close()  # release wkva_s space

            # ------- phase B+C: kv_b (use_ag) interleaved with attention -----
            with (
                tc.tile_pool(name="lg2", bufs=2) as lg2,
                tc.tile_pool(name="mp", bufs=3) as mp,
                tc.tile_pool(name="pb", bufs=2) as pb,
                tc.tile_pool(name="dvp", bufs=2) as dvp,
                tc.tile_pool(name="ost", bufs=3) as ost,
                tc.tile_pool(name="psatt", bufs=4, space="PSUM") as psatt,
                tc.tile_pool(name="psden", bufs=2, space="PSUM") as psden,
                tc.tile_pool(name="pso", bufs=2, space="PSUM") as pso,
                tc.tile_pool(name="drC", bufs=2, space="DRAM") as drC,
            ):
                def kvb(sb):
                    ssl = slice(sb * SB, (sb + 1) * SB)
                    # normalized latent + roped k_pe from the AllGather
                    lg_t = lg2.tile([128, NL, SB], BF16, tag="lat")
                    nc.sync.dma_start(
                        lg_t[:],
                        lat_full_d[sb, 0:L, :].rearrange("(lt p) s -> p lt s", p=128))
                    nc.sync.dma_start(k0[0:64, ssl], lat_full_d[sb, L:L + ROPE, :])
                    nc.vector.tensor_copy(k1[0:64, ssl], k0[0:64, ssl])

                    kbp = psatt.tile([128, SB], F32, tag="att")
                    for lt in range(NL):
                        nc.tensor.matmul(kbp[:], wkvbk_s[:, lt, :], lg_t[:, lt, :],
                                         start=(lt == 0), stop=(lt == NL - 1))
                    nc.vector.tensor_copy(k0[64:128, ssl], kbp[0:64, :])
                    nc.vector.tensor_copy(k1[64:128, ssl], kbp[64:128, :])

                    for st in range(SB // 128):
                        vp = psatt.tile([128, VH * HPC], F32, tag="att")
                        for lt in range(NL):
                            nc.tensor.matmul(vp[:],
                                             lg_t[:, lt, st * 128:(st + 1) * 128],
                                             wkvbv_s[:, lt, :],
                                             start=(lt == 0), stop=(lt == NL - 1))
                        nc.scalar.copy(v_sb[:, sb * 4 + st, :], vp[:])

                def attn(qb):
                    for h in range(HPC):
                        kh = k0 if h == 0 else k1
                        qsl = slice(qb * SB, (qb + 1) * SB)
                        active = [ki for ki in range(NKT) if not skip[ki][qb]]
                        probsT = pb.tile([128, NKT, SB], BF16, tag="probsT")
                        for ki in active:
                            ap_ = psatt.tile([128, SB], F32, tag="att")
                            has_mask = add[ki][qb]
                            nc.tensor.matmul(ap_[:], kh[:, ki * 128:(ki + 1) * 128],
                                             q_all[:, h, qsl],
                                             start=True, stop=not has_mask)
                            if has_mask:
                                m_t = mp.tile([128, SB], BF16, tag="mask")
                                nc.sync.dma_start(
                                    m_t[:], maskT_d[ki * 128:(ki + 1) * 128, qsl])
                                nc.tensor.matmul(ap_[:], ident[:], m_t[:],
                                                 start=False, stop=True)
                            nc.scalar.activation(probsT[:, ki, :], ap_[:],
                                                 mybir.ActivationFunctionType.Exp,
                                                 scale=SCALE)
                        dp = psden.tile([1, SB], F32, tag="den")
                        for i, ki in enumerate(active):
                            nc.tensor.matmul(dp[:], ones_c[:], probsT[:, ki, :],
                                             start=(i == 0), stop=(i == len(active) - 1))
                        dinv = dvp.tile([1, SB], F32, tag="dinv")
                        nc.vector.reciprocal(dinv[:], dp[:])
                        dv_dram = drC.tile([1, SB], F32, tag="dvd")
                        nc.sync.dma_start(dv_dram[:], dinv[:])
                        dinvb = dvp.tile([128, SB], F32, tag="dinvb")
                        nc.sync.dma_start(dinvb[:], _bcast128(dv_dram[:], SB))
                        op_ = pso.tile([128, SB], F32, tag="o")
                        for i, ki in enumerate(active):
                            nc.tensor.matmul(op_[:], v_sb[:, ki, h * VH:(h + 1) * VH],
                                             probsT[:, ki, :],
                                             start=(i == 0), stop=(i == len(active) - 1))
                        nc.vector.tensor_mul(attn_T[:, h, qsl], op_[:], dinvb[:])

                if use_ag:
                    # run each attention q-block as soon as the kv_b s-blocks
                    # it needs are done (causal: qb right after sb == qb)
                    needed = []
                    for qb in range(NSB):
                        act = [ki for ki in range(NKT) if not skip[ki][qb]]
                        needed.append((max(act) // (SB // 128)) if act else -1)
                    for sb in range(NSB):
                        kvb(sb)
                        for qb in range(NSB):
                            if needed[qb] == sb:
                                attn(qb)
                    for qb in range(NSB):
                        if needed[qb] < 0:
                            nc.vector.memset(attn_T[:, :, qb * SB:(qb + 1) * SB], 0.0)
                else:
                    for qb in range(NSB):
                        attn(qb)

                # ---------------- phase D: output projection -----------------
                # s2 outer so each s-block's wo work is ready as soon as its
                # attention blocks finish -> overlaps the attention tail.
                for s2 in range(NSB):
                    for mt in range(D // 128):
                        wp = psatt.tile([128, SB], F32, tag="att")
                        for cc in range(HPC):
                            nc.tensor.matmul(wp[:], wo_s[:, cc, mt * 128:(mt + 1) * 128],
                                             attn_T[:, cc, s2 * SB:(s2 + 1) * SB],
                                             start=(cc == 0), stop=(cc == HPC - 1))
                        o_t = ost.tile([128, SB], F32, tag="ostage")
                        nc.any.tensor_copy(out=o_t[:], in_=wp[:])
                        nc.sync.dma_start(
                            out_d[mt * 128:(mt + 1) * 128, s2 * SB:(s2 + 1) * SB],
                            o_t[:])

    nc.compile()
    return nc, need_mask


def kernel(x, cos, sin, mask, wq, wkv_a, kv_norm_w, wkv_b, wo, start_pos=0):
    x = np.asarray(x, np.float32)
    cos = np.asarray(cos, np.float32)
    sin = np.asarray(sin, np.float32)
    mask = np.asarray(mask, np.float32)
    wq = np.asarray(wq, np.float32)
    wkv_a = np.asarray(wkv_a, np.float32)
    kv_norm_w = np.asarray(kv_norm_w, np.float32)
    wkv_b = np.asarray(wkv_b, np.float32)
    wo = np.asarray(wo, np.float32)

    # mask block metadata: [qb, qi, kt, kj]
    mr = mask.reshape(NSB, SB, NKT, 128)
    skip_qk = (mr <= -1e8).all(axis=(1, 3))          # [qb, kt]
    nonzero_qk = (mr != 0).any(axis=(1, 3))          # [qb, kt]
    skip = skip_qk.T.copy()                          # [kt, qb]
    add = (nonzero_qk & ~skip_qk).T.copy()
    key = (bool(USE_AG), skip.tobytes(), add.tobytes())
    if key not in _BUILD_CACHE:
        _BUILD_CACHE[key] = _build(skip, add, USE_AG)
    nc, need_mask = _BUILD_CACHE[key]

    # ---- host-side shard prep ----
    deint = np.concatenate([np.arange(0, ROPE, 2), np.arange(1, ROPE, 2)])
    wq_h = wq.reshape(H, 128, D)
    # per-head row order [rope deinterleaved; nope]
    qrows = np.concatenate([wq_h[:, NOPE + deint, :], wq_h[:, 0:NOPE, :]], axis=1)
    wkva_perm = np.concatenate([wkv_a[0:L], wkv_a[L + deint]], axis=0)
    wkvb_h = wkv_b.reshape(H, NOPE + VH, L)

    xT = np.ascontiguousarray(x[0].T).astype(NPBF16)
    cosT = np.ascontiguousarray(cos.T)
    sinT = np.ascontiguousarray(sin.T)
    wkvaT = np.ascontiguousarray(wkva_perm.T).astype(NPBF16)
    shared = {"xT": xT, "cosT": cosT, "sinT": sinT, "wkvaT": wkvaT}
    if need_mask:
        shared["maskT"] = np.ascontiguousarray(mask.T * (1.0 / SCALE)).astype(NPBF16)

    in_maps = []
    for c in range(NCORES):
        hs = [HPC * c + i for i in range(HPC)]
        wqT_c = np.ascontiguousarray(
            qrows[hs].reshape(128 * HPC, D).T).astype(NPBF16)
        k_rows = (wkvb_h[hs, 0:NOPE, :] * kv_norm_w[None, None, :]).reshape(
            NOPE * HPC, L)
        wkvbTk_c = np.ascontiguousarray(k_rows.T).astype(NPBF16)
        v_rows = wkvb_h[hs, NOPE:, :].reshape(VH * HPC, L)
        wkvbTv_c = np.ascontiguousarray(v_rows.T).astype(NPBF16)
        woT_c = np.ascontiguousarray(
            wo[:, hs[0] * VH:(hs[-1] + 1) * VH].T).astype(NPBF16)
        m = dict(shared)
        m.update({"wqT": wqT_c, "wkvbTk": wkvbTk_c, "wkvbTv": wkvbTv_c,
                  "woT": woT_c})
        if USE_AG:
            ssl = slice(c * SB, (c + 1) * SB)
            m["xTs"] = np.ascontiguousarray(xT[:, ssl])
            m["cosS"] = np.ascontiguousarray(cosT[:, ssl])
            m["sinS"] = np.ascontiguousarray(sinT[:, ssl])
        in_maps.append(m)

    trace = os.environ.get("KERNEL_TRACE", "0") == "1"
    if trace:
        _install_ntff_hook()
    global last_results
    last_results = run_bass_kernel_spmd(nc, in_maps, core_ids=list(range(NCORES)),
                                        trace=trace)
    total = np.zeros((D, S), np.float32)
    for r in last_results.results:
        total += r["out"]
    return np.ascontiguousarray(total.T)[None]


def _install_ntff_hook():
    """Register the axon NTFF profiling hook (used when KERNEL_TRACE=1)."""
    import types
    import ctypes
    import contextlib

    if "antenv.axon_hooks" in sys.modules:
        return
    try:
        so = ctypes.CDLL("/opt/axon/libaxon_pjrt.so")
        so.axon_start_nrt_profile
    except (OSError, AttributeError):
        return
    so.axon_start_nrt_profile.argtypes = [ctypes.POINTER(ctypes.c_int64),
                                          ctypes.c_size_t]
    so.axon_start_nrt_profile.restype = ctypes.c_int64
    so.axon_stop_nrt_profile.argtypes = [ctypes.c_char_p]
    so.axon_stop_nrt_profile.restype = ctypes.c_int64

    @contextlib.contextmanager
    def _hook(output_dir, device_ids):
        import jax
        jax.devices()
        if device_ids:
            ids = (ctypes.c_int64 * len(device_ids))(*device_ids)
            rc = so.axon_start_nrt_profile(ids, len(device_ids))
        else:
            rc = so.axon_start_nrt_profile(None, 0)
        if rc != 0:
            raise RuntimeError(f"axon_start_nrt_profile rc={rc}")
        try:
            yield
        finally:
            n = so.axon_stop_nrt_profile(str(output_dir).encode())
            if n < 0:
                raise RuntimeError(f"axon_stop_nrt_profile rc={n}")

    mod = types.ModuleType("antenv.axon_hooks")
    mod.get_axon_ntff_profile_hook = lambda: _hook
    mod.set_axon_ntff_profile_hook = lambda h: None
    sys.modules["antenv.axon_hooks"] = mod
